# revision 19
# baseline (speedup 1.0000x reference)
"""Trainium2 Bass kernel for nn_DSPSModel (8-core SPMD).

Sharding:
- Encoder: tokens sharded 256/core for LN/residual (sequence parallel);
  attention heads (2/core) and FFN columns (512/core) tensor parallel;
  AllGather h.T before QKV/FFN, ReduceScatter after Wo/W2.
- GRU scan -> halo'd Jacobi fixed-point iteration: each core iterates a
  384-token block (128-token halo) with batched matmul sweeps in
  transposed layout; error contracts ~0.72x per sweep; no cross-core
  traffic during sweeps.  h_{-1}=0 boundary enforced via cmask0.
- Downstream (cls / parent scores / relation) sharded over query rows;
  one fused AllGather shares h_seq rows + root + keys.T + PM.T blocks.
- Big matmuls in float32r (tf32-like precision, full PE rate at N>=256).
"""
import sys
sys.path.insert(0, "/opt/trn_rl_repo")
import numpy as np
import concourse.bass as bass
import concourse.mybir as mybir
import concourse.tile as tile
from contextlib import ExitStack
from concourse import bacc
from concourse.bass_utils import run_bass_kernel_spmd
from concourse.masks import make_identity

F32 = mybir.dt.float32
F32R = mybir.dt.float32r
I32 = mybir.dt.int32
AX = mybir.AxisListType
OP = mybir.AluOpType
AF = mybir.ActivationFunctionType

L, D, H, NL, C, R = 2048, 1024, 16, 4, 64, 32
DH = D // H
NC_ = 8
TOK = L // NC_            # 256 tokens per core
BLK = TOK + 128           # 384-token GRU jacobi block (128-token halo)
KSWEEPS = 32
EPS_PAR = 1e-8
LN_EPS = 1e-5
SCALE = 1.0 / float(np.sqrt(DH))
NEG = -1e9

# AG4 per-rank block layout (flat fp32 elements, viewed as rows of 1024):
HSEQ_E = 0                       # h_seq own rows   [256, 1024]
ROOT_E = TOK * D                 # root row         [1, 1024]
KEYS_E = (TOK + 1) * D           # keys.T ext       8 blocks of [128, 257]
PM_E = KEYS_E + 8 * 128 * 257    # PM.T own         [64, 256]
AG4_ROWS = (PM_E + C * TOK) // D  # = 530

ts = bass.ts


def _ln_tile(nc, sb, x_ap, tag, eps_ap=None):
    """LayerNorm along the free dim of a [128, D] fp32 tile, in place."""
    s = sb.tile([128, 1], F32, tag=f"{tag}_s")
    nc.vector.tensor_reduce(s[:], x_ap, AX.X, OP.add)
    m = sb.tile([128, 1], F32, tag=f"{tag}_m")
    nc.scalar.mul(m[:], s[:], 1.0 / D)
    sq = sb.tile([128, D], F32, tag=f"{tag}_sq")
    ss = sb.tile([128, 1], F32, tag=f"{tag}_ss")
    nc.scalar.activation(sq[:], x_ap, AF.Square, accum_out=ss[:])
    m2 = sb.tile([128, 1], F32, tag=f"{tag}_m2")
    nc.scalar.square(m2[:], m[:])
    v = sb.tile([128, 1], F32, tag=f"{tag}_v")
    nc.vector.tensor_scalar(v[:], ss[:], 1.0 / D, m2[:], OP.mult, OP.subtract)
    sd = sb.tile([128, 1], F32, tag=f"{tag}_sd")
    nc.scalar.activation(sd[:], v[:], AF.Sqrt, bias=eps_ap)
    rstd = sb.tile([128, 1], F32, tag=f"{tag}_r")
    nc.vector.reciprocal(rstd[:], sd[:])
    nc.vector.tensor_scalar(x_ap, x_ap, m[:], rstd[:], OP.subtract, OP.mult)


def build_program():
    nc = bacc.Bacc("TRN2", target_bir_lowering=False, debug=False,
                   num_devices=NC_, enable_asserts=False)

    # ---------------- DRAM I/O ----------------
    x_in = nc.dram_tensor("x_c", [TOK, D], F32, kind="ExternalInput")
    wq, wk, wv, wo, w1, w2 = [], [], [], [], [], []
    for l in range(NL):
        wq.append(nc.dram_tensor(f"wqT_{l}", [D, 128], F32R, kind="ExternalInput"))
        wk.append(nc.dram_tensor(f"wkT_{l}", [D, 128], F32R, kind="ExternalInput"))
        wv.append(nc.dram_tensor(f"wvT_{l}", [D, 128], F32R, kind="ExternalInput"))
        wo.append(nc.dram_tensor(f"woT_{l}", [128, D], F32R, kind="ExternalInput"))
        w1.append(nc.dram_tensor(f"w1T_{l}", [D, 512], F32R, kind="ExternalInput"))
        w2.append(nc.dram_tensor(f"w2T_{l}", [512, D], F32R, kind="ExternalInput"))
    clswT = nc.dram_tensor("clswT", [D, C], F32R, kind="ExternalInput")
    mcp_in = nc.dram_tensor("mcp", [C + 2, C], F32R, kind="ExternalInput")
    mcp_last = nc.dram_tensor("mcp_last", [C, 1], F32R, kind="ExternalInput")
    wihT = nc.dram_tensor("wihT", [D, 3 * D], F32R, kind="ExternalInput")
    whhT_in = nc.dram_tensor("whhT", [D, 3 * D], F32R, kind="ExternalInput")
    wqfT = nc.dram_tensor("wqfT", [D, D], F32R, kind="ExternalInput")
    wkfT = nc.dram_tensor("wkfT", [D, D], F32R, kind="ExternalInput")
    rw1T = nc.dram_tensor("rw1T", [2 * D, D], F32R, kind="ExternalInput")
    rw2T = nc.dram_tensor("rw2T", [D, R], F32R, kind="ExternalInput")
    maskb = nc.dram_tensor("maskb", [TOK, L + 1], F32, kind="ExternalInput")
    haloofs = nc.dram_tensor("haloofs", [128, 8], I32, kind="ExternalInput")
    pvofs = nc.dram_tensor("pvofs", [128, 2], I32, kind="ExternalInput")
    cmask0 = nc.dram_tensor("cmask0", [128, 1], F32, kind="ExternalInput")

    o_cls = nc.dram_tensor("o_cls", [C, TOK], F32, kind="ExternalOutput")
    o_par = nc.dram_tensor("o_par", [TOK, L + 1], F32, kind="ExternalOutput")
    o_rel = nc.dram_tensor("o_rel", [R, TOK], F32, kind="ExternalOutput")

    # internal DRAM bounces
    agh_in = [nc.dram_tensor(f"agh_in{i}", [D, TOK], F32R) for i in range(2 * NL)]
    agh_out = [nc.dram_tensor(f"agh_out{i}", [NC_, D, TOK], F32R, addr_space="Shared")
               for i in range(2 * NL)]
    rs_in = [nc.dram_tensor(f"rs_in{i}", [L, D], F32) for i in range(2 * NL)]
    rs_out = [nc.dram_tensor(f"rs_out{i}", [TOK, D], F32) for i in range(2 * NL)]
    root_b = nc.dram_tensor("root_b", [D], F32)
    root_o = nc.dram_tensor("root_o", [D], F32, addr_space="Shared")
    agx_in = nc.dram_tensor("agx_in", [D, 128], F32R)
    agx_out = nc.dram_tensor("agx_out", [NC_, D, 128], F32R, addr_space="Shared")
    xstar_d = nc.dram_tensor("xstar_d", [D, TOK], F32R)
    ag4_in = nc.dram_tensor("ag4_in", [AG4_ROWS, D], F32)
    ag4_out = nc.dram_tensor("ag4_out", [NC_, AG4_ROWS, D], F32, addr_space="Shared")

    rg = [list(range(NC_))]
    ag4i_flat = ag4_in[:].rearrange("a b -> (a b)")

    with tile.TileContext(nc) as tc:
        with (
            tc.tile_pool(name="const", bufs=1) as cpool,
            tc.tile_pool(name="psumA", bufs=3, space="PSUM") as ps,
            tc.tile_pool(name="psumB", bufs=1, space="PSUM") as psb,
        ):
            ident = cpool.tile([128, 128], F32)
            make_identity(nc, ident[:])
            eps_ln = cpool.tile([128, 1], F32)
            nc.vector.memset(eps_ln[:], LN_EPS)
            eps_par = cpool.tile([128, 1], F32)
            nc.vector.memset(eps_par[:], EPS_PAR)

            _enc_stack = ExitStack()
            ep = _enc_stack.enter_context(tc.tile_pool(name="enc", bufs=1))
            ew = _enc_stack.enter_context(tc.tile_pool(name="encw", bufs=2))
            h_sh = ep.tile([128, 2, D], F32, tag="h_sh")
            hTfull = ep.tile([128, NC_, L], F32R, tag="hTfull")
            hT_sh = ep.tile([128, NC_, TOK], F32R, tag="hT_sh")

            # ---------- fuse LN ----------
            for m in range(2):
                nc.sync.dma_start(h_sh[:, m, :], x_in[ts(m, 128), :])
                _ln_tile(nc, ew, h_sh[:, m, :], "ln", eps_ln[:])

            def transpose_shard(dst_sb):
                for m in range(2):
                    for k in range(8):
                        pt = ps.tile([128, 128], F32, tag="mm")
                        nc.tensor.transpose(pt[:], h_sh[:, m, ts(k, 128)], ident[:])
                        nc.scalar.copy(dst_sb[:, k, ts(m, 128)], pt[:])

            def ag_layer(i):
                nc.sync.dma_start(agh_in[i][:].rearrange("(k p) t -> p k t", p=128),
                                  hT_sh[:])
                nc.gpsimd.collective_compute(
                    "AllGather", OP.bypass, ins=[agh_in[i][:]], outs=[agh_out[i][:]],
                    replica_groups=rg)

            def load_hTfull(i):
                for k in range(8):
                    nc.sync.dma_start(
                        hTfull[:, k, :].rearrange("p (r t) -> p r t", r=NC_),
                        agh_out[i][:, ts(k, 128), :].rearrange("r p t -> p r t"))

            transpose_shard(hT_sh)
            ag_layer(0)

            # ---------------- encoder layers ----------------
            for l in range(NL):
                with tc.tile_pool(name=f"lay{l}", bufs=1) as lp, \
                     tc.tile_pool(name=f"l2a_{l}", bufs=2) as lw2, \
                     tc.tile_pool(name=f"l2b_{l}", bufs=2) as lwb:
                    wq_s = lp.tile([128, 8, 128], F32R, tag="wq")
                    wk_s = lp.tile([128, 8, 128], F32R, tag="wk")
                    wv_s = lp.tile([128, 8, 128], F32R, tag="wv")
                    wo_h0 = lp.tile([64, D], F32R, tag="wo0")
                    wo_h1 = lp.tile([64, D], F32R, tag="wo1")
                    wo_h = [wo_h0, wo_h1]
                    nc.sync.dma_start(wq_s[:], wq[l][:].rearrange("(k p) m -> p k m", p=128))
                    nc.sync.dma_start(wk_s[:], wk[l][:].rearrange("(k p) m -> p k m", p=128))
                    nc.sync.dma_start(wv_s[:], wv[l][:].rearrange("(k p) m -> p k m", p=128))
                    nc.sync.dma_start(wo_h0[:], wo[l][0:64, :])
                    nc.sync.dma_start(wo_h1[:], wo[l][64:128, :])
                    load_hTfull(2 * l)

                    # k.T (both heads, all tokens) + v_ext
                    kT = lp.tile([128, L], F32R, tag="kT")
                    v_ext = lp.tile([128, 16, 132], F32R, tag="v_ext")
                    nc.vector.memset(v_ext[:].bitcast(F32), 1.0)
                    for nchk in range(4):
                        pq = ps.tile([128, 512], F32, tag="mm")
                        for k in range(8):
                            nc.tensor.matmul(pq[:], wk_s[:, k, :],
                                             hTfull[:, k, ts(nchk, 512)],
                                             start=(k == 0), stop=(k == 7))
                        nc.scalar.copy(kT[:, ts(nchk, 512)], pq[:])
                        pq2 = ps.tile([128, 512], F32, tag="mm")
                        for k in range(8):
                            nc.tensor.matmul(pq2[:], wv_s[:, k, :],
                                             hTfull[:, k, ts(nchk, 512)],
                                             start=(k == 0), stop=(k == 7))
                        vev = lw2.tile([128, 512], F32, tag="ev")
                        nc.scalar.copy(vev[:], pq2[:])
                        for i4 in range(4):
                            kt = 4 * nchk + i4
                            pv = ps.tile([128, 128], F32, tag="mm")
                            nc.tensor.transpose(pv[:], vev[:, ts(i4, 128)], ident[:])
                            nc.scalar.copy(v_ext[:, kt, 0:64], pv[:, 0:64])
                            nc.scalar.copy(v_ext[:, kt, 66:130], pv[:, 64:128])

                    # attention + fused Wo, per 512-token query chunk
                    for qc in range(4):
                        qTc = lw2.tile([128, 512], F32R, tag="qTc")
                        pq = ps.tile([128, 512], F32, tag="mm")
                        for k in range(8):
                            nc.tensor.matmul(pq[:], wq_s[:, k, :],
                                             hTfull[:, k, ts(qc, 512)],
                                             start=(k == 0), stop=(k == 7))
                        nc.scalar.copy(qTc[:], pq[:])
                        oTc0 = lw2.tile([64, 512], F32R, tag="oTc0")
                        oTc1 = lw2.tile([64, 512], F32R, tag="oTc1")
                        oTc = [oTc0, oTc1]
                        for h in range(2):
                            po = psb.tile([66, 512], F32, tag="pv")
                            for half in range(4):
                                es = [lp.tile([128, 512], F32R, tag=f"es{i}",
                                              name=f"es{i}") for i in range(4)]
                                for i in range(4):
                                    kt = 4 * half + i
                                    pst = ps.tile([128, 512], F32, tag="mm")
                                    nc.tensor.matmul(
                                        pst[:], kT[ts(h, 64), ts(kt, 128)],
                                        qTc[ts(h, 64), :], start=True, stop=True)
                                    nc.scalar.activation(es[i][:], pst[:], AF.Exp,
                                                         scale=SCALE)
                                for i in range(4):
                                    kt = 4 * half + i
                                    nc.tensor.matmul(po[:], v_ext[:, kt, ts(h, 66)],
                                                     es[i][:], start=(kt == 0),
                                                     stop=(kt == 15))
                            zhi = lw2.tile([66, 512], F32, tag="zhi")
                            nc.scalar.copy(zhi[64:65, :], po[64:65, :])
                            zr = lw2.tile([1, 512], F32, tag="zr")
                            nc.sync.dma_start(zr[:], zhi[64:65, :])
                            nc.vector.reciprocal(zr[:], zr[:])
                            zrb = lw2.tile([64, 512], F32, tag="zrb")
                            nc.gpsimd.partition_broadcast(zrb[:], zr[:])
                            nc.vector.tensor_tensor(oTc[h][:], po[0:64, :], zrb[:],
                                                    OP.mult)
                        for tt4 in range(4):
                            tt = 4 * qc + tt4
                            for nck in range(2):
                                pa = ps.tile([128, 512], F32, tag="mm")
                                for h in range(2):
                                    nc.tensor.matmul(pa[:], oTc[h][:, ts(tt4, 128)],
                                                     wo_h[h][:, ts(nck, 512)],
                                                     start=(h == 0), stop=(h == 1))
                                sa = lw2.tile([128, 512], F32, tag="ev")
                                nc.vector.tensor_copy(sa[:], pa[:])
                                nc.sync.dma_start(
                                    rs_in[2 * l][ts(tt, 128), ts(nck, 512)], sa[:])
                    nc.gpsimd.collective_compute(
                        "ReduceScatter", OP.add, ins=[rs_in[2 * l][:]],
                        outs=[rs_out[2 * l][:]], replica_groups=rg)
                    for m in range(2):
                        radd = ew.tile([128, D], F32, tag="radd")
                        nc.sync.dma_start(radd[:], rs_out[2 * l][ts(m, 128), :])
                        nc.vector.tensor_tensor(h_sh[:, m, :], h_sh[:, m, :],
                                                radd[:], OP.add)
                        _ln_tile(nc, ew, h_sh[:, m, :], "ln", eps_ln[:])
                    transpose_shard(hT_sh)
                    ag_layer(2 * l + 1)
                    load_hTfull(2 * l + 1)

                    # FFN (W1/W2 streamed; f1T holds a quarter of tokens)
                    f1T = lp.tile([128, 4, L // 4], F32R, tag="f1T")
                    for tc2 in range(4):
                        for m in range(4):
                            w1c = lwb.tile([128, 8, 128], F32R, tag="w1c")
                            nc.sync.dma_start(
                                w1c[:],
                                w1[l][:, ts(m, 128)].rearrange("(k p) f -> p k f", p=128))
                            pf = ps.tile([128, 512], F32, tag="mm")
                            for k in range(8):
                                nc.tensor.matmul(pf[:], w1c[:, k, :],
                                                 hTfull[:, k, ts(tc2, 512)],
                                                 start=(k == 0), stop=(k == 7))
                            nc.scalar.activation(f1T[:, m, :], pf[:], AF.Gelu)
                        for nck in range(2):
                            w2c = lwb.tile([128, 4, 512], F32R, tag="w2c")
                            nc.sync.dma_start(
                                w2c[:],
                                w2[l][:, ts(nck, 512)].rearrange("(k p) f -> p k f", p=128))
                            for tt4 in range(4):
                                tt = 4 * tc2 + tt4
                                pf2 = ps.tile([128, 512], F32, tag="mm")
                                for k4 in range(4):
                                    nc.tensor.matmul(pf2[:], f1T[:, k4, ts(tt4, 128)],
                                                     w2c[:, k4, :],
                                                     start=(k4 == 0), stop=(k4 == 3))
                                sf = lw2.tile([128, 512], F32, tag="ev")
                                nc.vector.tensor_copy(sf[:], pf2[:])
                                nc.sync.dma_start(
                                    rs_in[2 * l + 1][ts(tt, 128), ts(nck, 512)], sf[:])
                    nc.gpsimd.collective_compute(
                        "ReduceScatter", OP.add, ins=[rs_in[2 * l + 1][:]],
                        outs=[rs_out[2 * l + 1][:]], replica_groups=rg)
                    for m in range(2):
                        radd = ew.tile([128, D], F32, tag="radd")
                        nc.sync.dma_start(radd[:], rs_out[2 * l + 1][ts(m, 128), :])
                        nc.vector.tensor_tensor(h_sh[:, m, :], h_sh[:, m, :],
                                                radd[:], OP.add)
                        _ln_tile(nc, ew, h_sh[:, m, :], "ln", eps_ln[:])
                    if l < NL - 1:
                        transpose_shard(hT_sh)
                        ag_layer(2 * l + 2)

            # ============ x_star: h_sh (normal), xstarT (transposed) ============
            transpose_shard(hT_sh)
            nc.sync.dma_start(xstar_d[:].rearrange("(k p) t -> p k t", p=128),
                              hT_sh[:])
            nc.sync.dma_start(agx_in[:].rearrange("(k p) t -> p k t", p=128),
                              hT_sh[:, :, 128:256])
            _enc_stack.close()
            _mid_stack = ExitStack()
            pp = _mid_stack.enter_context(tc.tile_pool(name="mid", bufs=1))
            mw = _mid_stack.enter_context(tc.tile_pool(name="midw", bufs=2))
            xstarT = pp.tile([128, NC_, TOK], F32R, tag="xstarT")
            nc.sync.dma_start(xstarT[:],
                              xstar_d[:].rearrange("(k p) t -> p k t", p=128))

            # ---- cls logits + softmax ----
            clsw_s = pp.tile([128, 8, C], F32R, tag="clsw")
            nc.sync.dma_start(clsw_s[:], clswT[:].rearrange("(k p) m -> p k m", p=128))
            pc = ps.tile([C, TOK], F32, tag="mm")
            for k in range(8):
                nc.tensor.matmul(pc[:], clsw_s[:, k, :], xstarT[:, k, :],
                                 start=(k == 0), stop=(k == 7))
            cls_sb = mw.tile([C, TOK], F32, tag="cls_sb")
            nc.vector.tensor_copy(cls_sb[:], pc[:])
            nc.sync.dma_start(o_cls[:], cls_sb[:])
            e_cls = pp.tile([C, TOK], F32R, tag="e_cls")
            nc.scalar.activation(e_cls[:], pc[:], AF.Exp)
            ones64 = cpool.tile([C, 1], F32R)
            nc.vector.memset(ones64[:].bitcast(F32), 1.0)
            pz = ps.tile([1, TOK], F32, tag="mm")
            nc.tensor.matmul(pz[:], ones64[:], e_cls[:], start=True, stop=True)
            zrc = mw.tile([1, TOK], F32, tag="zrc")
            nc.vector.reciprocal(zrc[:], pz[:])
            zrcb = mw.tile([C, TOK], F32, tag="zrcb")
            nc.gpsimd.partition_broadcast(zrcb[:], zrc[:])
            cprobT = pp.tile([C, TOK], F32R, tag="cprobT")
            nc.vector.tensor_tensor(cprobT[:], e_cls[:], zrcb[:], OP.mult)

            # ---- root mean via AllReduce ----
            rsum = mw.tile([128, 8, 1], F32, tag="rsum")
            nc.vector.tensor_reduce(rsum[:], xstarT[:].bitcast(F32), AX.X, OP.add)
            nc.sync.dma_start(root_b[:].rearrange("(k p) -> p k", p=128), rsum[:, :, 0])
            nc.gpsimd.collective_compute("AllReduce", OP.add, ins=[root_b[:]],
                                         outs=[root_o[:]], replica_groups=rg)
            root_s = pp.tile([128, 8, 1], F32R, tag="root_s")
            rtmp = mw.tile([128, 8], F32, tag="rtmp")
            nc.sync.dma_start(rtmp[:], root_o[:].rearrange("(k p) -> p k", p=128))
            nc.scalar.activation(root_s[:, :, 0], rtmp[:], AF.Copy, scale=1.0 / L)

            # ---- halo exchange of x_star.T ----
            nc.gpsimd.collective_compute("AllGather", OP.bypass, ins=[agx_in[:]],
                                         outs=[agx_out[:]], replica_groups=rg)
            hofs_s = pp.tile([128, 8], I32, tag="hofs")
            nc.sync.dma_start(hofs_s[:], haloofs[:])
            xhalo = pp.tile([128, 8, 128], F32R, tag="xhalo")
            for k in range(8):
                nc.gpsimd.indirect_dma_start(
                    out=xhalo[:, k, :], out_offset=None,
                    in_=agx_out[:].rearrange("r d t -> (r d) t"),
                    in_offset=bass.IndirectOffsetOnAxis(ap=hofs_s[:, k:k + 1], axis=0))

            # ============ GRU ============
            with tc.tile_pool(name="gruo", bufs=1) as go:
                hts = [go.tile([128, 8, BLK + 1], F32R, tag=f"ht{i}", name=f"ht{i}")
                       for i in range(2)]
                nc.vector.memset(hts[0][:].bitcast(F32), 0.0)
                nc.vector.memset(hts[1][:].bitcast(F32), 0.0)
                htf = hts[KSWEEPS % 2]

                with tc.tile_pool(name="gru", bufs=1) as gp, \
                     tc.tile_pool(name="gsc", bufs=2) as gsc, \
                     tc.tile_pool(name="gw", bufs=1) as gw:
                    giT = gp.tile([128, 24, BLK], F32, tag="giT")
                    for jb in range(8):
                        wcol = gw.tile([128, 8, 384], F32R, tag="wcol")
                        nc.sync.dma_start(
                            wcol[:],
                            wihT[:, ts(jb, 384)].rearrange("(k p) m -> p k m", p=128))
                        for mj in range(3):
                            pg = ps.tile([128, BLK], F32, tag="mm")
                            for k in range(8):
                                nc.tensor.matmul(pg[:, 0:128], wcol[:, k, ts(mj, 128)],
                                                 xhalo[:, k, :],
                                                 start=(k == 0), stop=(k == 7))
                            for k in range(8):
                                nc.tensor.matmul(pg[:, 128:BLK], wcol[:, k, ts(mj, 128)],
                                                 xstarT[:, k, :],
                                                 start=(k == 0), stop=(k == 7))
                            nc.scalar.copy(giT[:, 3 * jb + mj, :], pg[:])

                    whh_s = gp.tile([128, 8, 3 * D], F32R, tag="whh_s")
                    nc.sync.dma_start(whh_s[:],
                                      whhT_in[:].rearrange("(k p) m -> p k m", p=128))
                    cm0 = cpool.tile([128, 1], F32)
                    nc.sync.dma_start(cm0[:], cmask0[:])

                    for s in range(KSWEEPS):
                        ha, hb = hts[s % 2], hts[(s + 1) % 2]
                        for kk in range(8):
                            pgs = []
                            for g in range(3):
                                mj = 8 * g + kk
                                pgg = psb.tile([128, BLK], F32, tag=f"gh{g}", name=f"gh{g}")
                                for k in range(8):
                                    nc.tensor.matmul(pgg[:], whh_s[:, k, ts(mj, 128)],
                                                     ha[:, k, 0:BLK],
                                                     start=(k == 0), stop=(k == 7))
                                pgs.append(pgg)
                            r_t = gsc.tile([128, BLK], F32, tag="r_t")
                            nc.vector.tensor_tensor(r_t[:], pgs[0][:], giT[:, kk, :],
                                                    OP.add)
                            nc.scalar.activation(r_t[:], r_t[:], AF.Sigmoid)
                            z_t = gsc.tile([128, BLK], F32, tag="z_t")
                            nc.vector.tensor_tensor(z_t[:], pgs[1][:],
                                                    giT[:, 8 + kk, :], OP.add)
                            nc.scalar.activation(z_t[:], z_t[:], AF.Sigmoid)
                            n_t = gsc.tile([128, BLK], F32, tag="n_t")
                            nc.vector.tensor_tensor(n_t[:], r_t[:], pgs[2][:], OP.mult)
                            nc.vector.tensor_tensor(n_t[:], n_t[:],
                                                    giT[:, 16 + kk, :], OP.add)
                            nc.scalar.activation(n_t[:], n_t[:], AF.Tanh)
                            d_t = gsc.tile([128, BLK], F32, tag="d_t")
                            nc.vector.tensor_tensor(d_t[:], ha[:, kk, 0:BLK].bitcast(F32),
                                                    n_t[:], OP.subtract)
                            nc.vector.tensor_tensor(d_t[:], z_t[:], d_t[:], OP.mult)
                            nc.vector.tensor_tensor(hb[:, kk, 1:BLK + 1], n_t[:],
                                                    d_t[:], OP.add)
                        nc.vector.tensor_scalar_mul(hb[:, :, 128:129],
                                                    hb[:, :, 128:129], cm0[:])

                # ---------- build AG4 block (gru pools closed) ----------
                with tc.tile_pool(name="post", bufs=1) as gp, \
                     tc.tile_pool(name="postw", bufs=2) as gw2:
                    hnorm = gp.tile([128, 2, D], F32, tag="hnorm")
                    for m in range(2):
                        for k in range(8):
                            pt = ps.tile([128, 128], F32, tag="mm")
                            nc.tensor.transpose(
                                pt[:],
                                htf[:, k, 129 + 128 * m:129 + 128 * (m + 1)].bitcast(F32),
                                ident[:])
                            nc.scalar.copy(hnorm[:, m, ts(k, 128)], pt[:])
                        nc.sync.dma_start(ag4_in[ts(m, 128), :], hnorm[:, m, :])
                    nc.sync.dma_start(
                        ag4_in[TOK:TOK + 1, :].rearrange("o (k p) -> p (k o)", p=128),
                        root_s[:, :, 0].bitcast(F32))
                    # keys.T ext: [root | h own] through Wk
                    krhs = gp.tile([128, 8, 260], F32R, tag="krhs")
                    nc.vector.memset(krhs[:].bitcast(F32), 0.0)
                    nc.scalar.copy(krhs[:, :, 0:1], root_s[:])
                    nc.scalar.copy(krhs[:, :, 1:257], htf[:, :, 129:BLK + 1])
                    wkf_s = gp.tile([128, 8, D], F32R, tag="wkf_s")
                    nc.sync.dma_start(wkf_s[:],
                                      wkfT[:].rearrange("(k p) m -> p k m", p=128))
                    for m in range(8):
                        pk = ps.tile([128, 260], F32, tag="mm")
                        for k in range(8):
                            nc.tensor.matmul(pk[:], wkf_s[:, k, ts(m, 128)], krhs[:, k, :],
                                             start=(k == 0), stop=(k == 7))
                        sk = gw2.tile([128, 257], F32, tag="sk")
                        nc.vector.tensor_copy(sk[:], pk[:, 0:257])
                        nc.sync.dma_start(
                            ag4i_flat[KEYS_E + m * 128 * 257:
                                      KEYS_E + (m + 1) * 128 * 257]
                            .rearrange("(p j) -> p j", p=128), sk[:])
                    # PM.T own
                    mcp_s = cpool.tile([C + 2, C], F32R)
                    nc.sync.dma_start(mcp_s[:], mcp_in[:])
                    ppT = gp.tile([C + 2, TOK], F32R, tag="ppT")
                    nc.vector.memset(ppT[C:C + 2, :].bitcast(F32), 0.0)
                    nc.scalar.copy(ppT[0:C, :], cprobT[:])
                    pm = ps.tile([C, TOK], F32, tag="mm")
                    nc.tensor.matmul(pm[:], mcp_s[:], ppT[:], start=True, stop=True)
                    spm = gw2.tile([C, TOK], F32, tag="spm")
                    nc.vector.tensor_copy(spm[:], pm[:])
                    nc.sync.dma_start(
                        ag4i_flat[PM_E:PM_E + C * TOK].rearrange("(p j) -> p j", p=C),
                        spm[:])
                    nc.gpsimd.collective_compute(
                        "AllGather", OP.bypass, ins=[ag4_in[:]], outs=[ag4_out[:]],
                        replica_groups=rg)

                # ============ downstream ============
                with tc.tile_pool(name="down", bufs=1) as dp, \
                     tc.tile_pool(name="dw", bufs=2) as dw:
                    qT_o = dp.tile([128, 8, TOK], F32R, tag="qT_o")
                    for m in range(8):
                        wqf_s = dw.tile([128, 8, 128], F32R, tag="wqf_s")
                        nc.sync.dma_start(
                            wqf_s[:],
                            wqfT[:, ts(m, 128)].rearrange("(k p) mm -> p k mm", p=128))
                        pq = ps.tile([128, TOK], F32, tag="mm")
                        for k in range(8):
                            nc.tensor.matmul(pq[:], wqf_s[:, k, :],
                                             htf[:, k, 129:BLK + 1],
                                             start=(k == 0), stop=(k == 7))
                        nc.scalar.copy(qT_o[:, m, :], pq[:])

                    def keys_sec(r):
                        return (ag4_out[r][:].rearrange("a b -> (a b)")
                                [KEYS_E:KEYS_E + 8 * 128 * 257]
                                .rearrange("(m p j) -> p m j", p=128, j=257))

                    keysT = dp.tile([128, 8, L + 1], F32R, tag="keysT")
                    nc.sync.dma_start(keysT[:, :, 0:1].bitcast(F32),
                                      keys_sec(0)[:, :, 0:1])
                    for r_ in range(NC_):
                        nc.sync.dma_start(
                            keysT[:, :, 1 + TOK * r_:1 + TOK * (r_ + 1)].bitcast(F32),
                            keys_sec(r_)[:, :, 1:257])
                    pmT = dp.tile([C, L + 1], F32R, tag="pmT")
                    nc.sync.dma_start(pmT[:, 0:1], mcp_last[:])
                    for r_ in range(NC_):
                        nc.sync.dma_start(
                            pmT[:, 1 + TOK * r_:1 + TOK * (r_ + 1)].bitcast(F32),
                            ag4_out[r_][:].rearrange("a b -> (a b)")
                            [PM_E:PM_E + C * TOK].rearrange("(p j) -> p j", p=C))

                    mb_s = dp.tile([128, 2, L + 1], F32, tag="mb_s")
                    nc.sync.dma_start(mb_s[:],
                                      maskb[:].rearrange("(m p) j -> p m j", p=128))
                    nchunks = [(0, 512), (512, 512), (1024, 512), (1536, 512), (2047, 2)]
                    for m in range(2):
                        for (j0, jw) in nchunks:
                            pss = ps.tile([128, 512], F32, tag="mm")
                            for k in range(8):
                                nc.tensor.matmul(pss[:, 0:jw], qT_o[:, k, ts(m, 128)],
                                                 keysT[:, k, j0:j0 + jw],
                                                 start=(k == 0), stop=(k == 7))
                            psp = psb.tile([128, 512], F32, tag="pr")
                            nc.tensor.matmul(psp[:, 0:jw], cprobT[:, ts(m, 128)],
                                             pmT[:, j0:j0 + jw], start=True, stop=True)
                            lp_t = dw.tile([128, 512], F32, tag="lp_t")
                            nc.scalar.activation(lp_t[:, 0:jw], psp[:, 0:jw], AF.Ln,
                                                 bias=eps_par[:])
                            nc.vector.tensor_tensor(lp_t[:, 0:jw], lp_t[:, 0:jw],
                                                    pss[:, 0:jw], OP.add)
                            nc.vector.tensor_tensor(lp_t[:, 0:jw], lp_t[:, 0:jw],
                                                    mb_s[:, m, j0:j0 + jw], OP.add)
                            nc.sync.dma_start(o_par[ts(m, 128), j0:j0 + jw],
                                              lp_t[:, 0:jw])

                    # relation head
                    pvo_s = dp.tile([128, 2], I32, tag="pvo_s")
                    nc.sync.dma_start(pvo_s[:], pvofs[:])
                    pvn = dp.tile([128, 2, D], F32, tag="pvn")
                    for m in range(2):
                        nc.gpsimd.indirect_dma_start(
                            out=pvn[:, m, :], out_offset=None,
                            in_=ag4_out[:].rearrange("r b d -> (r b) d"),
                            in_offset=bass.IndirectOffsetOnAxis(
                                ap=pvo_s[:, m:m + 1], axis=0))
                    pvT = dp.tile([128, 8, TOK], F32R, tag="pvT")
                    for m in range(2):
                        for k in range(8):
                            pt = ps.tile([128, 128], F32, tag="mm")
                            nc.tensor.transpose(pt[:], pvn[:, m, ts(k, 128)], ident[:])
                            nc.scalar.copy(pvT[:, k, ts(m, 128)], pt[:])
                    rel1 = dp.tile([128, 8, TOK], F32R, tag="rel1")
                    rw1cols = []
                    for m in range(8):
                        rw1_s = dw.tile([128, 16, 128], F32R, tag="rw1_s")
                        nc.sync.dma_start(
                            rw1_s[:],
                            rw1T[:, ts(m, 128)].rearrange("(k p) mm -> p k mm", p=128))
                        pr1 = ps.tile([128, TOK], F32, tag="mm")
                        for k in range(16):
                            rhs = htf[:, k, 129:BLK + 1] if k < 8 else pvT[:, k - 8, :]
                            nc.tensor.matmul(pr1[:], rw1_s[:, k, :], rhs,
                                             start=(k == 0), stop=(k == 15))
                        nc.scalar.activation(rel1[:, m, :], pr1[:], AF.Gelu)
                    rw2_s = dw.tile([128, 8, R], F32R, tag="rw2_s")
                    nc.sync.dma_start(rw2_s[:],
                                      rw2T[:].rearrange("(k p) m -> p k m", p=128))
                    pr2 = ps.tile([R, TOK], F32, tag="mm")
                    for k in range(8):
                        nc.tensor.matmul(pr2[:], rw2_s[:, k, :], rel1[:, k, :],
                                         start=(k == 0), stop=(k == 7))
                    sr2 = dw.tile([R, TOK], F32, tag="sr2")
                    nc.vector.tensor_copy(sr2[:], pr2[:])
                    nc.sync.dma_start(o_rel[:], sr2[:])

            _mid_stack.close()

    nc.finalize()
    return nc


_PROG = None


def _get_prog():
    global _PROG
    if _PROG is None:
        _PROG = build_program()
    return _PROG


def _shard_inputs(inputs):
    f = lambda a: np.ascontiguousarray(np.asarray(a, dtype=np.float32))
    x = f(inputs["x"])
    Wqkv = f(inputs["enc_Wqkv"]); Wo = f(inputs["enc_Wo"])
    W1 = f(inputs["enc_W1"]); W2 = f(inputs["enc_W2"])
    clsW = f(inputs["cls_W"]); Mcp = f(inputs["M_cp"])
    Wih = f(inputs["gru_Wih"]); Whh = f(inputs["gru_Whh"])
    Wq = f(inputs["Wq"]); Wk = f(inputs["Wk"])
    rW1 = f(inputs["rel_W1"]); rW2 = f(inputs["rel_W2"])
    y = np.asarray(inputs["y_parent"]).astype(np.int64)

    shared = {
        "clswT": np.ascontiguousarray(clsW.T),
        "mcp": np.pad(Mcp, ((0, 1), (0, 0))),
        "mcp_last": np.ascontiguousarray(Mcp[C][:, None]),
        "wihT": np.ascontiguousarray(Wih.T),
        "whhT": np.ascontiguousarray(Whh.T),
        "wqfT": np.ascontiguousarray(Wq.T),
        "wkfT": np.ascontiguousarray(Wk.T),
        "rw1T": np.ascontiguousarray(rW1.T),
        "rw2T": np.ascontiguousarray(rW2.T),
    }
    in_maps = []
    rows = np.arange(TOK)
    jj = np.arange(L + 1)
    k_ = np.arange(8)
    p_ = np.arange(128)
    for c in range(NC_):
        m = dict(shared)
        sl = slice(128 * c, 128 * (c + 1))
        fl = slice(512 * c, 512 * (c + 1))
        m["x_c"] = np.ascontiguousarray(x[TOK * c:TOK * (c + 1)])
        for l in range(NL):
            Wq_l = Wqkv[l][0:D]; Wk_l = Wqkv[l][D:2 * D]; Wv_l = Wqkv[l][2 * D:3 * D]
            m[f"wqT_{l}"] = np.ascontiguousarray(Wq_l[sl].T)
            m[f"wkT_{l}"] = np.ascontiguousarray(Wk_l[sl].T)
            m[f"wvT_{l}"] = np.ascontiguousarray(Wv_l[sl].T)
            m[f"woT_{l}"] = np.ascontiguousarray(Wo[l][:, sl].T)
            m[f"w1T_{l}"] = np.ascontiguousarray(W1[l][fl].T)
            m[f"w2T_{l}"] = np.ascontiguousarray(W2[l][:, fl].T)
        t_glob = TOK * c + rows
        m["maskb"] = np.ascontiguousarray(
            np.where(jj[None, :] <= t_glob[:, None], 0.0, NEG).astype(np.float32))
        prev = (c - 1) % NC_
        m["haloofs"] = np.ascontiguousarray(
            (prev * D + (k_[None, :] * 128 + p_[:, None])).astype(np.int32))
        yc = y[TOK * c:TOK * (c + 1)]
        ycl = np.clip(yc, 0, L - 1)
        pvrow = (ycl // TOK) * AG4_ROWS + (ycl % TOK)
        pvrow = np.where(yc < 0, TOK, pvrow)  # -> root row of rank 0
        m["pvofs"] = np.ascontiguousarray(pvrow.reshape(2, 128).T.astype(np.int32))
        m["cmask0"] = np.full((128, 1), 0.0 if c == 0 else 1.0, np.float32)
        in_maps.append(m)
    return in_maps


def kernel(**inputs):
    prog = _get_prog()
    in_maps = _shard_inputs(inputs)
    res = run_bass_kernel_spmd(prog, in_maps, list(range(NC_))).results
    cls_logits = np.concatenate([res[c]["o_cls"].T for c in range(NC_)], 0)
    par = np.concatenate([res[c]["o_par"] for c in range(NC_)], 0)
    rel = np.concatenate([res[c]["o_rel"].T for c in range(NC_)], 0)
    jj = np.arange(L + 1)
    tt_ = np.arange(L)
    par = np.where(jj[None, :] <= tt_[:, None], par,
                   np.float32(NEG)).astype(np.float32)
    return cls_logits, par, rel


def time_device(inputs, iters=6):
    """Best-effort device execution timing: inputs staged on device once,
    min wall time of repeated sharded executions (includes PJRT dispatch)."""
    import time
    import jax
    import numpy as _np
    from jax.sharding import Mesh, PartitionSpec
    from jax.experimental.shard_map import shard_map
    from concourse import bass2jax as b2j

    nc = _get_prog()
    in_maps = _shard_inputs(inputs)
    b2j.install_neuronx_cc_hook()
    partition_name = nc.partition_id_tensor.name if nc.partition_id_tensor else None
    import concourse.mybir as _mb
    in_names, out_names, out_avals, zero_outs = [], [], [], []
    for alloc in nc.m.functions[0].allocations:
        if not isinstance(alloc, _mb.MemoryLocationSet):
            continue
        name = alloc.memorylocations[0].name
        if alloc.kind == "ExternalInput":
            if name != partition_name:
                in_names.append(name)
        elif alloc.kind == "ExternalOutput":
            shape = tuple(alloc.tensor_shape)
            dtype = _mb.dt.np(alloc.dtype)
            out_names.append(name)
            out_avals.append(jax.core.ShapedArray(shape, dtype))
            zero_outs.append(_np.zeros(shape, dtype))
    n_params = len(in_names)
    n_outs = len(out_avals)
    in_names_all = in_names + out_names
    if partition_name is not None:
        in_names_all.append(partition_name)

    def _body(*args):
        operands = list(args)
        if partition_name is not None:
            operands.append(b2j.partition_id_tensor())
        outs = b2j._bass_exec_p.bind(
            *operands, out_avals=tuple(out_avals), in_names=tuple(in_names_all),
            out_names=tuple(out_names), lowering_input_output_aliases=(),
            sim_require_finite=True, sim_require_nnan=True, nc=nc)
        return tuple(outs)

    devices = jax.devices()[:NC_]
    mesh = Mesh(_np.asarray(devices), ("core",))
    in_specs = (PartitionSpec("core"),) * (n_params + n_outs)
    out_specs = (PartitionSpec("core"),) * n_outs
    sharded = jax.jit(shard_map(_body, mesh=mesh, in_specs=in_specs,
                                out_specs=out_specs, check_rep=False),
                      keep_unused=True)
    concat_in = [_np.concatenate([_np.asarray(in_maps[c][nm]) for c in range(NC_)], 0)
                 for nm in in_names]
    concat_zero = [_np.concatenate([z] * NC_, 0) for z in zero_outs]
    args = [jax.device_put(a) for a in concat_in + concat_zero]
    best = float("inf")
    for _ in range(iters):
        t0 = time.perf_counter()
        outs = sharded(*args)
        jax.block_until_ready(outs)
        best = min(best, time.perf_counter() - t0)
    return best


# revision 21
# speedup vs baseline: 1.1702x; 1.1702x over previous
"""Trainium2 Bass kernel for nn_DSPSModel (8-core SPMD).

Sharding:
- Encoder: tokens sharded 256/core for LN/residual (sequence parallel);
  attention heads (2/core) and FFN columns (512/core) tensor parallel;
  AllGather h.T before QKV/FFN, ReduceScatter after Wo/W2.
- GRU scan -> halo'd Jacobi fixed-point iteration: each core iterates a
  384-token block (128-token halo) with batched matmul sweeps in
  transposed layout; error contracts ~0.72x per sweep; no cross-core
  traffic during sweeps.  h_{-1}=0 boundary enforced via cmask0.
- Downstream (cls / parent scores / relation) sharded over query rows;
  one fused AllGather shares h_seq rows + root + keys.T + PM.T blocks.
- Big matmuls in float32r (tf32-like precision, full PE rate at N>=256).
"""
import sys
sys.path.insert(0, "/opt/trn_rl_repo")
import numpy as np
import concourse.bass as bass
import concourse.mybir as mybir
import concourse.tile as tile
from contextlib import ExitStack
from concourse import bacc
from concourse.bass_utils import run_bass_kernel_spmd
from concourse.masks import make_identity

F32 = mybir.dt.float32
F32R = mybir.dt.float32r
I32 = mybir.dt.int32
AX = mybir.AxisListType
OP = mybir.AluOpType
AF = mybir.ActivationFunctionType

L, D, H, NL, C, R = 2048, 1024, 16, 4, 64, 32
DH = D // H
NC_ = 8
TOK = L // NC_            # 256 tokens per core
BLK = TOK + 128           # 384-token GRU jacobi block (128-token halo)
import os as _os
KSWEEPS = int(_os.environ.get("GRU_SWEEPS", "40"))
NL_BUILD = int(_os.environ.get("NL_BUILD", "4"))
EPS_PAR = 1e-8
LN_EPS = 1e-5
SCALE = 1.0 / float(np.sqrt(DH))
NEG = -1e9

# AG4 per-rank block layout (flat fp32 elements, viewed as rows of 1024):
HSEQ_E = 0                       # h_seq own rows   [256, 1024]
ROOT_E = TOK * D                 # root row         [1, 1024]
KEYS_E = (TOK + 1) * D           # keys.T ext       8 blocks of [128, 257]
PM_E = KEYS_E + 8 * 128 * 257    # PM.T own         [64, 256]
AG4_ROWS = (PM_E + C * TOK) // D  # = 530

ts = bass.ts


def _ln_tile(nc, sb, x_ap, tag, eps_ap=None):
    """LayerNorm along the free dim of a [128, D] fp32 tile, in place."""
    s = sb.tile([128, 1], F32, tag=f"{tag}_s")
    nc.vector.tensor_reduce(s[:], x_ap, AX.X, OP.add)
    m = sb.tile([128, 1], F32, tag=f"{tag}_m")
    nc.scalar.mul(m[:], s[:], 1.0 / D)
    sq = sb.tile([128, D], F32, tag=f"{tag}_sq")
    ss = sb.tile([128, 1], F32, tag=f"{tag}_ss")
    nc.scalar.activation(sq[:], x_ap, AF.Square, accum_out=ss[:])
    m2 = sb.tile([128, 1], F32, tag=f"{tag}_m2")
    nc.scalar.square(m2[:], m[:])
    v = sb.tile([128, 1], F32, tag=f"{tag}_v")
    nc.vector.tensor_scalar(v[:], ss[:], 1.0 / D, m2[:], OP.mult, OP.subtract)
    sd = sb.tile([128, 1], F32, tag=f"{tag}_sd")
    nc.scalar.activation(sd[:], v[:], AF.Sqrt, bias=eps_ap)
    rstd = sb.tile([128, 1], F32, tag=f"{tag}_r")
    nc.vector.reciprocal(rstd[:], sd[:])
    nc.vector.tensor_scalar(x_ap, x_ap, m[:], rstd[:], OP.subtract, OP.mult)


def build_program():
    nc = bacc.Bacc("TRN2", target_bir_lowering=False, debug=False,
                   num_devices=NC_, enable_asserts=False)

    # ---------------- DRAM I/O ----------------
    x_in = nc.dram_tensor("x_c", [TOK, D], F32, kind="ExternalInput")
    wq, wk, wv, wo, w1, w2 = [], [], [], [], [], []
    for l in range(NL):
        wq.append(nc.dram_tensor(f"wqT_{l}", [D, 128], F32R, kind="ExternalInput"))
        wk.append(nc.dram_tensor(f"wkT_{l}", [D, 128], F32R, kind="ExternalInput"))
        wv.append(nc.dram_tensor(f"wvT_{l}", [D, 128], F32R, kind="ExternalInput"))
        wo.append(nc.dram_tensor(f"woT_{l}", [128, D], F32R, kind="ExternalInput"))
        w1.append(nc.dram_tensor(f"w1T_{l}", [D, 512], F32R, kind="ExternalInput"))
        w2.append(nc.dram_tensor(f"w2T_{l}", [512, D], F32R, kind="ExternalInput"))
    clswT = nc.dram_tensor("clswT", [D, C], F32R, kind="ExternalInput")
    mcp_in = nc.dram_tensor("mcp", [C + 2, C], F32R, kind="ExternalInput")
    mcp_last = nc.dram_tensor("mcp_last", [C, 1], F32R, kind="ExternalInput")
    wihT = nc.dram_tensor("wihT", [D, 3 * D], F32R, kind="ExternalInput")
    whhT_in = nc.dram_tensor("whhT", [D, 3 * D], F32R, kind="ExternalInput")
    wqfT = nc.dram_tensor("wqfT", [D, D], F32R, kind="ExternalInput")
    wkfT = nc.dram_tensor("wkfT", [D, D], F32R, kind="ExternalInput")
    rw1T = nc.dram_tensor("rw1T", [2 * D, D], F32R, kind="ExternalInput")
    rw2T = nc.dram_tensor("rw2T", [D, R], F32R, kind="ExternalInput")
    maskb = nc.dram_tensor("maskb", [TOK, L + 1], F32, kind="ExternalInput")
    haloofs = nc.dram_tensor("haloofs", [128, 8], I32, kind="ExternalInput")
    pvofs = nc.dram_tensor("pvofs", [128, 2], I32, kind="ExternalInput")
    cmask0 = nc.dram_tensor("cmask0", [128, 1], F32, kind="ExternalInput")

    o_cls = nc.dram_tensor("o_cls", [C, TOK], F32, kind="ExternalOutput")
    o_par = nc.dram_tensor("o_par", [TOK, L + 1], F32, kind="ExternalOutput")
    o_rel = nc.dram_tensor("o_rel", [R, TOK], F32, kind="ExternalOutput")

    # internal DRAM bounces
    agh_in = [nc.dram_tensor(f"agh_in{i}", [D, TOK], F32R) for i in range(2 * NL)]
    agh_out = [nc.dram_tensor(f"agh_out{i}", [NC_, D, TOK], F32R, addr_space="Shared")
               for i in range(2 * NL)]
    rs_in = [nc.dram_tensor(f"rs_in{i}", [L, D], F32) for i in range(2 * NL)]
    rs_out = [nc.dram_tensor(f"rs_out{i}", [TOK, D], F32) for i in range(2 * NL)]
    root_b = nc.dram_tensor("root_b", [D], F32)
    root_o = nc.dram_tensor("root_o", [D], F32, addr_space="Shared")
    agx_in = nc.dram_tensor("agx_in", [D, 128], F32R)
    agx_out = nc.dram_tensor("agx_out", [NC_, D, 128], F32R, addr_space="Shared")
    xstar_d = nc.dram_tensor("xstar_d", [D, TOK], F32R)
    ag4_in = nc.dram_tensor("ag4_in", [AG4_ROWS, D], F32)
    ag4_out = nc.dram_tensor("ag4_out", [NC_, AG4_ROWS, D], F32, addr_space="Shared")

    rg = [list(range(NC_))]
    ag4i_flat = ag4_in[:].rearrange("a b -> (a b)")

    with tile.TileContext(nc) as tc:
        with (
            tc.tile_pool(name="const", bufs=1) as cpool,
            tc.tile_pool(name="psumA", bufs=3, space="PSUM") as ps,
            tc.tile_pool(name="psumB", bufs=1, space="PSUM") as psb,
        ):
            ident = cpool.tile([128, 128], F32)
            make_identity(nc, ident[:])
            eps_ln = cpool.tile([128, 1], F32)
            nc.vector.memset(eps_ln[:], LN_EPS)
            eps_par = cpool.tile([128, 1], F32)
            nc.vector.memset(eps_par[:], EPS_PAR)

            _enc_stack = ExitStack()
            ep = _enc_stack.enter_context(tc.tile_pool(name="enc", bufs=1))
            ew = _enc_stack.enter_context(tc.tile_pool(name="encw", bufs=2))
            h_sh = ep.tile([128, 2, D], F32, tag="h_sh")
            hTfull = ep.tile([128, NC_, L], F32R, tag="hTfull")
            hT_sh = ep.tile([128, NC_, TOK], F32R, tag="hT_sh")

            # ---------- fuse LN ----------
            for m in range(2):
                nc.sync.dma_start(h_sh[:, m, :], x_in[ts(m, 128), :])
                _ln_tile(nc, ew, h_sh[:, m, :], "ln", eps_ln[:])

            def transpose_shard(dst_sb):
                for m in range(2):
                    for k in range(8):
                        pt = ps.tile([128, 128], F32, tag="mm")
                        nc.tensor.transpose(pt[:], h_sh[:, m, ts(k, 128)], ident[:])
                        nc.scalar.copy(dst_sb[:, k, ts(m, 128)], pt[:])

            def ag_layer(i):
                nc.sync.dma_start(agh_in[i][:].rearrange("(k p) t -> p k t", p=128),
                                  hT_sh[:])
                nc.gpsimd.collective_compute(
                    "AllGather", OP.bypass, ins=[agh_in[i][:]], outs=[agh_out[i][:]],
                    replica_groups=rg)

            def load_hTfull(i):
                for k in range(8):
                    nc.sync.dma_start(
                        hTfull[:, k, :].rearrange("p (r t) -> p r t", r=NC_),
                        agh_out[i][:, ts(k, 128), :].rearrange("r p t -> p r t"))

            transpose_shard(hT_sh)
            ag_layer(0)

            # ---------------- encoder layers ----------------
            for l in range(NL_BUILD):
                with tc.tile_pool(name=f"lay{l}", bufs=1) as lp, \
                     tc.tile_pool(name=f"l2a_{l}", bufs=2) as lw2, \
                     tc.tile_pool(name=f"l2b_{l}", bufs=2) as lwb:
                    wq_s = lp.tile([128, 8, 128], F32R, tag="wq")
                    wk_s = lp.tile([128, 8, 128], F32R, tag="wk")
                    wv_s = lp.tile([128, 8, 128], F32R, tag="wv")
                    wo_h0 = lp.tile([64, D], F32R, tag="wo0")
                    wo_h1 = lp.tile([64, D], F32R, tag="wo1")
                    wo_h = [wo_h0, wo_h1]
                    nc.sync.dma_start(wq_s[:], wq[l][:].rearrange("(k p) m -> p k m", p=128))
                    nc.sync.dma_start(wk_s[:], wk[l][:].rearrange("(k p) m -> p k m", p=128))
                    nc.sync.dma_start(wv_s[:], wv[l][:].rearrange("(k p) m -> p k m", p=128))
                    nc.sync.dma_start(wo_h0[:], wo[l][0:64, :])
                    nc.sync.dma_start(wo_h1[:], wo[l][64:128, :])
                    load_hTfull(2 * l)

                    # k.T (both heads, all tokens) + v_ext
                    kT = lp.tile([128, L], F32R, tag="kT")
                    v_ext = lp.tile([128, 16, 132], F32R, tag="v_ext")
                    nc.vector.memset(v_ext[:].bitcast(F32), 1.0)
                    for nchk in range(4):
                        pq = ps.tile([128, 512], F32, tag="mm")
                        for k in range(8):
                            nc.tensor.matmul(pq[:], wk_s[:, k, :],
                                             hTfull[:, k, ts(nchk, 512)],
                                             start=(k == 0), stop=(k == 7))
                        nc.scalar.copy(kT[:, ts(nchk, 512)], pq[:])
                        pq2 = ps.tile([128, 512], F32, tag="mm")
                        for k in range(8):
                            nc.tensor.matmul(pq2[:], wv_s[:, k, :],
                                             hTfull[:, k, ts(nchk, 512)],
                                             start=(k == 0), stop=(k == 7))
                        vev = lw2.tile([128, 512], F32, tag="ev")
                        nc.scalar.copy(vev[:], pq2[:])
                        for i4 in range(4):
                            kt = 4 * nchk + i4
                            pv = ps.tile([128, 128], F32, tag="mm")
                            nc.tensor.transpose(pv[:], vev[:, ts(i4, 128)], ident[:])
                            nc.scalar.copy(v_ext[:, kt, 0:64], pv[:, 0:64])
                            nc.scalar.copy(v_ext[:, kt, 66:130], pv[:, 64:128])

                    # attention + fused Wo, per 512-token query chunk
                    for qc in range(4):
                        qTc = lw2.tile([128, 512], F32R, tag="qTc")
                        pq = ps.tile([128, 512], F32, tag="mm")
                        for k in range(8):
                            nc.tensor.matmul(pq[:], wq_s[:, k, :],
                                             hTfull[:, k, ts(qc, 512)],
                                             start=(k == 0), stop=(k == 7))
                        nc.scalar.copy(qTc[:], pq[:])
                        oTc0 = lw2.tile([64, 512], F32R, tag="oTc0")
                        oTc1 = lw2.tile([64, 512], F32R, tag="oTc1")
                        oTc = [oTc0, oTc1]
                        for h in range(2):
                            po = psb.tile([66, 512], F32, tag="pv")
                            for half in range(4):
                                es = [lp.tile([128, 512], F32R, tag=f"es{i}",
                                              name=f"es{i}") for i in range(4)]
                                for i in range(4):
                                    kt = 4 * half + i
                                    pst = ps.tile([128, 512], F32, tag="mm")
                                    nc.tensor.matmul(
                                        pst[:], kT[ts(h, 64), ts(kt, 128)],
                                        qTc[ts(h, 64), :], start=True, stop=True)
                                    nc.scalar.activation(es[i][:], pst[:], AF.Exp,
                                                         scale=SCALE)
                                for i in range(4):
                                    kt = 4 * half + i
                                    nc.tensor.matmul(po[:], v_ext[:, kt, ts(h, 66)],
                                                     es[i][:], start=(kt == 0),
                                                     stop=(kt == 15))
                            zhi = lw2.tile([66, 512], F32, tag="zhi")
                            nc.scalar.copy(zhi[64:65, :], po[64:65, :])
                            zr = lw2.tile([1, 512], F32, tag="zr")
                            nc.sync.dma_start(zr[:], zhi[64:65, :])
                            nc.vector.reciprocal(zr[:], zr[:])
                            zrb = lw2.tile([64, 512], F32, tag="zrb")
                            nc.gpsimd.partition_broadcast(zrb[:], zr[:])
                            nc.vector.tensor_tensor(oTc[h][:], po[0:64, :], zrb[:],
                                                    OP.mult)
                        for tt4 in range(4):
                            tt = 4 * qc + tt4
                            for nck in range(2):
                                pa = ps.tile([128, 512], F32, tag="mm")
                                for h in range(2):
                                    nc.tensor.matmul(pa[:], oTc[h][:, ts(tt4, 128)],
                                                     wo_h[h][:, ts(nck, 512)],
                                                     start=(h == 0), stop=(h == 1))
                                sa = lw2.tile([128, 512], F32, tag="ev")
                                nc.vector.tensor_copy(sa[:], pa[:])
                                nc.sync.dma_start(
                                    rs_in[2 * l][ts(tt, 128), ts(nck, 512)], sa[:])
                    nc.gpsimd.collective_compute(
                        "ReduceScatter", OP.add, ins=[rs_in[2 * l][:]],
                        outs=[rs_out[2 * l][:]], replica_groups=rg)
                    for m in range(2):
                        radd = ew.tile([128, D], F32, tag="radd")
                        nc.sync.dma_start(radd[:], rs_out[2 * l][ts(m, 128), :])
                        nc.vector.tensor_tensor(h_sh[:, m, :], h_sh[:, m, :],
                                                radd[:], OP.add)
                        _ln_tile(nc, ew, h_sh[:, m, :], "ln", eps_ln[:])
                    transpose_shard(hT_sh)
                    ag_layer(2 * l + 1)
                    load_hTfull(2 * l + 1)

                    # FFN (W1/W2 streamed; f1T holds a quarter of tokens)
                    f1T = lp.tile([128, 4, L // 4], F32R, tag="f1T")
                    for tc2 in range(4):
                        for m in range(4):
                            w1c = lwb.tile([128, 8, 128], F32R, tag="w1c")
                            nc.sync.dma_start(
                                w1c[:],
                                w1[l][:, ts(m, 128)].rearrange("(k p) f -> p k f", p=128))
                            pf = ps.tile([128, 512], F32, tag="mm")
                            for k in range(8):
                                nc.tensor.matmul(pf[:], w1c[:, k, :],
                                                 hTfull[:, k, ts(tc2, 512)],
                                                 start=(k == 0), stop=(k == 7))
                            nc.scalar.activation(f1T[:, m, :], pf[:], AF.Gelu)
                        for nck in range(2):
                            w2c = lwb.tile([128, 4, 512], F32R, tag="w2c")
                            nc.sync.dma_start(
                                w2c[:],
                                w2[l][:, ts(nck, 512)].rearrange("(k p) f -> p k f", p=128))
                            for tt4 in range(4):
                                tt = 4 * tc2 + tt4
                                pf2 = ps.tile([128, 512], F32, tag="mm")
                                for k4 in range(4):
                                    nc.tensor.matmul(pf2[:], f1T[:, k4, ts(tt4, 128)],
                                                     w2c[:, k4, :],
                                                     start=(k4 == 0), stop=(k4 == 3))
                                sf = lw2.tile([128, 512], F32, tag="ev")
                                nc.vector.tensor_copy(sf[:], pf2[:])
                                nc.sync.dma_start(
                                    rs_in[2 * l + 1][ts(tt, 128), ts(nck, 512)], sf[:])
                    nc.gpsimd.collective_compute(
                        "ReduceScatter", OP.add, ins=[rs_in[2 * l + 1][:]],
                        outs=[rs_out[2 * l + 1][:]], replica_groups=rg)
                    for m in range(2):
                        radd = ew.tile([128, D], F32, tag="radd")
                        nc.sync.dma_start(radd[:], rs_out[2 * l + 1][ts(m, 128), :])
                        nc.vector.tensor_tensor(h_sh[:, m, :], h_sh[:, m, :],
                                                radd[:], OP.add)
                        _ln_tile(nc, ew, h_sh[:, m, :], "ln", eps_ln[:])
                    if l < NL_BUILD - 1:
                        transpose_shard(hT_sh)
                        ag_layer(2 * l + 2)

            # ============ x_star: h_sh (normal), xstarT (transposed) ============
            transpose_shard(hT_sh)
            nc.sync.dma_start(xstar_d[:].rearrange("(k p) t -> p k t", p=128),
                              hT_sh[:])
            nc.sync.dma_start(agx_in[:].rearrange("(k p) t -> p k t", p=128),
                              hT_sh[:, :, 128:256])
            _enc_stack.close()
            _mid_stack = ExitStack()
            pp = _mid_stack.enter_context(tc.tile_pool(name="mid", bufs=1))
            mw = _mid_stack.enter_context(tc.tile_pool(name="midw", bufs=2))
            xstarT = pp.tile([128, NC_, TOK], F32R, tag="xstarT")
            nc.sync.dma_start(xstarT[:],
                              xstar_d[:].rearrange("(k p) t -> p k t", p=128))

            # ---- cls logits + softmax ----
            clsw_s = pp.tile([128, 8, C], F32R, tag="clsw")
            nc.sync.dma_start(clsw_s[:], clswT[:].rearrange("(k p) m -> p k m", p=128))
            pc = ps.tile([C, TOK], F32, tag="mm")
            for k in range(8):
                nc.tensor.matmul(pc[:], clsw_s[:, k, :], xstarT[:, k, :],
                                 start=(k == 0), stop=(k == 7))
            cls_sb = mw.tile([C, TOK], F32, tag="cls_sb")
            nc.vector.tensor_copy(cls_sb[:], pc[:])
            nc.sync.dma_start(o_cls[:], cls_sb[:])
            e_cls = pp.tile([C, TOK], F32R, tag="e_cls")
            nc.scalar.activation(e_cls[:], pc[:], AF.Exp)
            ones64 = cpool.tile([C, 1], F32R)
            nc.vector.memset(ones64[:].bitcast(F32), 1.0)
            pz = ps.tile([1, TOK], F32, tag="mm")
            nc.tensor.matmul(pz[:], ones64[:], e_cls[:], start=True, stop=True)
            zrc = mw.tile([1, TOK], F32, tag="zrc")
            nc.vector.reciprocal(zrc[:], pz[:])
            zrcb = mw.tile([C, TOK], F32, tag="zrcb")
            nc.gpsimd.partition_broadcast(zrcb[:], zrc[:])
            cprobT = pp.tile([C, TOK], F32R, tag="cprobT")
            nc.vector.tensor_tensor(cprobT[:], e_cls[:], zrcb[:], OP.mult)

            # ---- root mean via AllReduce ----
            rsum = mw.tile([128, 8, 1], F32, tag="rsum")
            nc.vector.tensor_reduce(rsum[:], xstarT[:].bitcast(F32), AX.X, OP.add)
            nc.sync.dma_start(root_b[:].rearrange("(k p) -> p k", p=128), rsum[:, :, 0])
            nc.gpsimd.collective_compute("AllReduce", OP.add, ins=[root_b[:]],
                                         outs=[root_o[:]], replica_groups=rg)
            root_s = pp.tile([128, 8, 1], F32R, tag="root_s")
            rtmp = mw.tile([128, 8], F32, tag="rtmp")
            nc.sync.dma_start(rtmp[:], root_o[:].rearrange("(k p) -> p k", p=128))
            nc.scalar.activation(root_s[:, :, 0], rtmp[:], AF.Copy, scale=1.0 / L)

            # ---- halo exchange of x_star.T ----
            nc.gpsimd.collective_compute("AllGather", OP.bypass, ins=[agx_in[:]],
                                         outs=[agx_out[:]], replica_groups=rg)
            hofs_s = pp.tile([128, 8], I32, tag="hofs")
            nc.sync.dma_start(hofs_s[:], haloofs[:])
            xhalo = pp.tile([128, 8, 128], F32R, tag="xhalo")
            for k in range(8):
                nc.gpsimd.indirect_dma_start(
                    out=xhalo[:, k, :], out_offset=None,
                    in_=agx_out[:].rearrange("r d t -> (r d) t"),
                    in_offset=bass.IndirectOffsetOnAxis(ap=hofs_s[:, k:k + 1], axis=0))

            # ============ GRU ============
            with tc.tile_pool(name="gruo", bufs=1) as go:
                hts = [go.tile([128, 8, BLK + 1], F32R, tag=f"ht{i}", name=f"ht{i}")
                       for i in range(2)]
                nc.vector.memset(hts[0][:].bitcast(F32), 0.0)
                nc.vector.memset(hts[1][:].bitcast(F32), 0.0)
                htf = hts[KSWEEPS % 2]

                with tc.tile_pool(name="gru", bufs=1) as gp, \
                     tc.tile_pool(name="gsc", bufs=2) as gsc, \
                     tc.tile_pool(name="gw", bufs=1) as gw:
                    giT = gp.tile([128, 24, BLK], F32, tag="giT")
                    for jb in range(8):
                        wcol = gw.tile([128, 8, 384], F32R, tag="wcol")
                        nc.sync.dma_start(
                            wcol[:],
                            wihT[:, ts(jb, 384)].rearrange("(k p) m -> p k m", p=128))
                        for mj in range(3):
                            pg = ps.tile([128, BLK], F32, tag="mm")
                            for k in range(8):
                                nc.tensor.matmul(pg[:, 0:128], wcol[:, k, ts(mj, 128)],
                                                 xhalo[:, k, :],
                                                 start=(k == 0), stop=(k == 7))
                            for k in range(8):
                                nc.tensor.matmul(pg[:, 128:BLK], wcol[:, k, ts(mj, 128)],
                                                 xstarT[:, k, :],
                                                 start=(k == 0), stop=(k == 7))
                            nc.scalar.copy(giT[:, 3 * jb + mj, :], pg[:])

                    whh_s = gp.tile([128, 8, 3 * D], F32R, tag="whh_s")
                    nc.sync.dma_start(whh_s[:],
                                      whhT_in[:].rearrange("(k p) m -> p k m", p=128))
                    cm0 = cpool.tile([128, 1], F32)
                    nc.sync.dma_start(cm0[:], cmask0[:])

                    for s in range(KSWEEPS):
                        ha, hb = hts[s % 2], hts[(s + 1) % 2]
                        for kk in range(8):
                            pgs = []
                            for g in range(3):
                                mj = 8 * g + kk
                                pgg = psb.tile([128, BLK], F32, tag=f"gh{g}", name=f"gh{g}")
                                for k in range(8):
                                    nc.tensor.matmul(pgg[:], whh_s[:, k, ts(mj, 128)],
                                                     ha[:, k, 0:BLK],
                                                     start=(k == 0), stop=(k == 7))
                                pgs.append(pgg)
                            r_t = gsc.tile([128, BLK], F32, tag="r_t")
                            nc.vector.tensor_tensor(r_t[:], pgs[0][:], giT[:, kk, :],
                                                    OP.add)
                            nc.scalar.activation(r_t[:], r_t[:], AF.Sigmoid)
                            z_t = gsc.tile([128, BLK], F32, tag="z_t")
                            nc.vector.tensor_tensor(z_t[:], pgs[1][:],
                                                    giT[:, 8 + kk, :], OP.add)
                            nc.scalar.activation(z_t[:], z_t[:], AF.Sigmoid)
                            n_t = gsc.tile([128, BLK], F32, tag="n_t")
                            nc.vector.tensor_tensor(n_t[:], r_t[:], pgs[2][:], OP.mult)
                            nc.vector.tensor_tensor(n_t[:], n_t[:],
                                                    giT[:, 16 + kk, :], OP.add)
                            nc.scalar.activation(n_t[:], n_t[:], AF.Tanh)
                            d_t = gsc.tile([128, BLK], F32, tag="d_t")
                            nc.vector.tensor_tensor(d_t[:], ha[:, kk, 0:BLK].bitcast(F32),
                                                    n_t[:], OP.subtract)
                            nc.vector.tensor_tensor(d_t[:], z_t[:], d_t[:], OP.mult)
                            nc.vector.tensor_tensor(hb[:, kk, 1:BLK + 1], n_t[:],
                                                    d_t[:], OP.add)
                        nc.vector.tensor_scalar_mul(hb[:, :, 128:129],
                                                    hb[:, :, 128:129], cm0[:])

                # ---------- build AG4 block (gru pools closed) ----------
                with tc.tile_pool(name="post", bufs=1) as gp, \
                     tc.tile_pool(name="postw", bufs=2) as gw2:
                    hnorm = gp.tile([128, 2, D], F32, tag="hnorm")
                    for m in range(2):
                        for k in range(8):
                            pt = ps.tile([128, 128], F32, tag="mm")
                            nc.tensor.transpose(
                                pt[:],
                                htf[:, k, 129 + 128 * m:129 + 128 * (m + 1)].bitcast(F32),
                                ident[:])
                            nc.scalar.copy(hnorm[:, m, ts(k, 128)], pt[:])
                        nc.sync.dma_start(ag4_in[ts(m, 128), :], hnorm[:, m, :])
                    nc.sync.dma_start(
                        ag4_in[TOK:TOK + 1, :].rearrange("o (k p) -> p (k o)", p=128),
                        root_s[:, :, 0].bitcast(F32))
                    # keys.T ext: [root | h own] through Wk
                    krhs = gp.tile([128, 8, 260], F32R, tag="krhs")
                    nc.vector.memset(krhs[:].bitcast(F32), 0.0)
                    nc.scalar.copy(krhs[:, :, 0:1], root_s[:])
                    nc.scalar.copy(krhs[:, :, 1:257], htf[:, :, 129:BLK + 1])
                    wkf_s = gp.tile([128, 8, D], F32R, tag="wkf_s")
                    nc.sync.dma_start(wkf_s[:],
                                      wkfT[:].rearrange("(k p) m -> p k m", p=128))
                    for m in range(8):
                        pk = ps.tile([128, 260], F32, tag="mm")
                        for k in range(8):
                            nc.tensor.matmul(pk[:], wkf_s[:, k, ts(m, 128)], krhs[:, k, :],
                                             start=(k == 0), stop=(k == 7))
                        sk = gw2.tile([128, 257], F32, tag="sk")
                        nc.vector.tensor_copy(sk[:], pk[:, 0:257])
                        nc.sync.dma_start(
                            ag4i_flat[KEYS_E + m * 128 * 257:
                                      KEYS_E + (m + 1) * 128 * 257]
                            .rearrange("(p j) -> p j", p=128), sk[:])
                    # PM.T own
                    mcp_s = cpool.tile([C + 2, C], F32R)
                    nc.sync.dma_start(mcp_s[:], mcp_in[:])
                    ppT = gp.tile([C + 2, TOK], F32R, tag="ppT")
                    nc.vector.memset(ppT[C:C + 2, :].bitcast(F32), 0.0)
                    nc.scalar.copy(ppT[0:C, :], cprobT[:])
                    pm = ps.tile([C, TOK], F32, tag="mm")
                    nc.tensor.matmul(pm[:], mcp_s[:], ppT[:], start=True, stop=True)
                    spm = gw2.tile([C, TOK], F32, tag="spm")
                    nc.vector.tensor_copy(spm[:], pm[:])
                    nc.sync.dma_start(
                        ag4i_flat[PM_E:PM_E + C * TOK].rearrange("(p j) -> p j", p=C),
                        spm[:])
                    nc.gpsimd.collective_compute(
                        "AllGather", OP.bypass, ins=[ag4_in[:]], outs=[ag4_out[:]],
                        replica_groups=rg)

                # ============ downstream ============
                with tc.tile_pool(name="down", bufs=1) as dp, \
                     tc.tile_pool(name="dw", bufs=2) as dw:
                    qT_o = dp.tile([128, 8, TOK], F32R, tag="qT_o")
                    for m in range(8):
                        wqf_s = dw.tile([128, 8, 128], F32R, tag="wqf_s")
                        nc.sync.dma_start(
                            wqf_s[:],
                            wqfT[:, ts(m, 128)].rearrange("(k p) mm -> p k mm", p=128))
                        pq = ps.tile([128, TOK], F32, tag="mm")
                        for k in range(8):
                            nc.tensor.matmul(pq[:], wqf_s[:, k, :],
                                             htf[:, k, 129:BLK + 1],
                                             start=(k == 0), stop=(k == 7))
                        nc.scalar.copy(qT_o[:, m, :], pq[:])

                    def keys_sec(r):
                        return (ag4_out[r][:].rearrange("a b -> (a b)")
                                [KEYS_E:KEYS_E + 8 * 128 * 257]
                                .rearrange("(m p j) -> p m j", p=128, j=257))

                    keysT = dp.tile([128, 8, L + 1], F32R, tag="keysT")
                    nc.sync.dma_start(keysT[:, :, 0:1].bitcast(F32),
                                      keys_sec(0)[:, :, 0:1])
                    for r_ in range(NC_):
                        nc.sync.dma_start(
                            keysT[:, :, 1 + TOK * r_:1 + TOK * (r_ + 1)].bitcast(F32),
                            keys_sec(r_)[:, :, 1:257])
                    pmT = dp.tile([C, L + 1], F32R, tag="pmT")
                    nc.sync.dma_start(pmT[:, 0:1], mcp_last[:])
                    for r_ in range(NC_):
                        nc.sync.dma_start(
                            pmT[:, 1 + TOK * r_:1 + TOK * (r_ + 1)].bitcast(F32),
                            ag4_out[r_][:].rearrange("a b -> (a b)")
                            [PM_E:PM_E + C * TOK].rearrange("(p j) -> p j", p=C))

                    mb_s = dp.tile([128, 2, L + 1], F32, tag="mb_s")
                    nc.sync.dma_start(mb_s[:],
                                      maskb[:].rearrange("(m p) j -> p m j", p=128))
                    nchunks = [(0, 512), (512, 512), (1024, 512), (1536, 512), (2047, 2)]
                    for m in range(2):
                        for (j0, jw) in nchunks:
                            pss = ps.tile([128, 512], F32, tag="mm")
                            for k in range(8):
                                nc.tensor.matmul(pss[:, 0:jw], qT_o[:, k, ts(m, 128)],
                                                 keysT[:, k, j0:j0 + jw],
                                                 start=(k == 0), stop=(k == 7))
                            psp = psb.tile([128, 512], F32, tag="pr")
                            nc.tensor.matmul(psp[:, 0:jw], cprobT[:, ts(m, 128)],
                                             pmT[:, j0:j0 + jw], start=True, stop=True)
                            lp_t = dw.tile([128, 512], F32, tag="lp_t")
                            nc.scalar.activation(lp_t[:, 0:jw], psp[:, 0:jw], AF.Ln,
                                                 bias=eps_par[:])
                            nc.vector.tensor_tensor(lp_t[:, 0:jw], lp_t[:, 0:jw],
                                                    pss[:, 0:jw], OP.add)
                            nc.vector.tensor_tensor(lp_t[:, 0:jw], lp_t[:, 0:jw],
                                                    mb_s[:, m, j0:j0 + jw], OP.add)
                            nc.sync.dma_start(o_par[ts(m, 128), j0:j0 + jw],
                                              lp_t[:, 0:jw])

                    # relation head
                    pvo_s = dp.tile([128, 2], I32, tag="pvo_s")
                    nc.sync.dma_start(pvo_s[:], pvofs[:])
                    pvn = dp.tile([128, 2, D], F32, tag="pvn")
                    for m in range(2):
                        nc.gpsimd.indirect_dma_start(
                            out=pvn[:, m, :], out_offset=None,
                            in_=ag4_out[:].rearrange("r b d -> (r b) d"),
                            in_offset=bass.IndirectOffsetOnAxis(
                                ap=pvo_s[:, m:m + 1], axis=0))
                    pvT = dp.tile([128, 8, TOK], F32R, tag="pvT")
                    for m in range(2):
                        for k in range(8):
                            pt = ps.tile([128, 128], F32, tag="mm")
                            nc.tensor.transpose(pt[:], pvn[:, m, ts(k, 128)], ident[:])
                            nc.scalar.copy(pvT[:, k, ts(m, 128)], pt[:])
                    rel1 = dp.tile([128, 8, TOK], F32R, tag="rel1")
                    rw1cols = []
                    for m in range(8):
                        rw1_s = dw.tile([128, 16, 128], F32R, tag="rw1_s")
                        nc.sync.dma_start(
                            rw1_s[:],
                            rw1T[:, ts(m, 128)].rearrange("(k p) mm -> p k mm", p=128))
                        pr1 = ps.tile([128, TOK], F32, tag="mm")
                        for k in range(16):
                            rhs = htf[:, k, 129:BLK + 1] if k < 8 else pvT[:, k - 8, :]
                            nc.tensor.matmul(pr1[:], rw1_s[:, k, :], rhs,
                                             start=(k == 0), stop=(k == 15))
                        nc.scalar.activation(rel1[:, m, :], pr1[:], AF.Gelu)
                    rw2_s = dw.tile([128, 8, R], F32R, tag="rw2_s")
                    nc.sync.dma_start(rw2_s[:],
                                      rw2T[:].rearrange("(k p) m -> p k m", p=128))
                    pr2 = ps.tile([R, TOK], F32, tag="mm")
                    for k in range(8):
                        nc.tensor.matmul(pr2[:], rw2_s[:, k, :], rel1[:, k, :],
                                         start=(k == 0), stop=(k == 7))
                    sr2 = dw.tile([R, TOK], F32, tag="sr2")
                    nc.vector.tensor_copy(sr2[:], pr2[:])
                    nc.sync.dma_start(o_rel[:], sr2[:])

            _mid_stack.close()

    nc.finalize()
    return nc


_PROG = None


def _get_prog():
    global _PROG
    if _PROG is None:
        _PROG = build_program()
    return _PROG


def _shard_inputs(inputs):
    f = lambda a: np.ascontiguousarray(np.asarray(a, dtype=np.float32))
    x = f(inputs["x"])
    Wqkv = f(inputs["enc_Wqkv"]); Wo = f(inputs["enc_Wo"])
    W1 = f(inputs["enc_W1"]); W2 = f(inputs["enc_W2"])
    clsW = f(inputs["cls_W"]); Mcp = f(inputs["M_cp"])
    Wih = f(inputs["gru_Wih"]); Whh = f(inputs["gru_Whh"])
    Wq = f(inputs["Wq"]); Wk = f(inputs["Wk"])
    rW1 = f(inputs["rel_W1"]); rW2 = f(inputs["rel_W2"])
    y = np.asarray(inputs["y_parent"]).astype(np.int64)

    shared = {
        "clswT": np.ascontiguousarray(clsW.T),
        "mcp": np.pad(Mcp, ((0, 1), (0, 0))),
        "mcp_last": np.ascontiguousarray(Mcp[C][:, None]),
        "wihT": np.ascontiguousarray(Wih.T),
        "whhT": np.ascontiguousarray(Whh.T),
        "wqfT": np.ascontiguousarray(Wq.T),
        "wkfT": np.ascontiguousarray(Wk.T),
        "rw1T": np.ascontiguousarray(rW1.T),
        "rw2T": np.ascontiguousarray(rW2.T),
    }
    in_maps = []
    rows = np.arange(TOK)
    jj = np.arange(L + 1)
    k_ = np.arange(8)
    p_ = np.arange(128)
    for c in range(NC_):
        m = dict(shared)
        sl = slice(128 * c, 128 * (c + 1))
        fl = slice(512 * c, 512 * (c + 1))
        m["x_c"] = np.ascontiguousarray(x[TOK * c:TOK * (c + 1)])
        for l in range(NL):
            Wq_l = Wqkv[l][0:D]; Wk_l = Wqkv[l][D:2 * D]; Wv_l = Wqkv[l][2 * D:3 * D]
            m[f"wqT_{l}"] = np.ascontiguousarray(Wq_l[sl].T)
            m[f"wkT_{l}"] = np.ascontiguousarray(Wk_l[sl].T)
            m[f"wvT_{l}"] = np.ascontiguousarray(Wv_l[sl].T)
            m[f"woT_{l}"] = np.ascontiguousarray(Wo[l][:, sl].T)
            m[f"w1T_{l}"] = np.ascontiguousarray(W1[l][fl].T)
            m[f"w2T_{l}"] = np.ascontiguousarray(W2[l][:, fl].T)
        t_glob = TOK * c + rows
        m["maskb"] = np.ascontiguousarray(
            np.where(jj[None, :] <= t_glob[:, None], 0.0, NEG).astype(np.float32))
        prev = (c - 1) % NC_
        m["haloofs"] = np.ascontiguousarray(
            (prev * D + (k_[None, :] * 128 + p_[:, None])).astype(np.int32))
        yc = y[TOK * c:TOK * (c + 1)]
        ycl = np.clip(yc, 0, L - 1)
        pvrow = (ycl // TOK) * AG4_ROWS + (ycl % TOK)
        pvrow = np.where(yc < 0, TOK, pvrow)  # -> root row of rank 0
        m["pvofs"] = np.ascontiguousarray(pvrow.reshape(2, 128).T.astype(np.int32))
        m["cmask0"] = np.full((128, 1), 0.0 if c == 0 else 1.0, np.float32)
        in_maps.append(m)
    return in_maps


def kernel(**inputs):
    prog = _get_prog()
    in_maps = _shard_inputs(inputs)
    res = run_bass_kernel_spmd(prog, in_maps, list(range(NC_))).results
    cls_logits = np.concatenate([res[c]["o_cls"].T for c in range(NC_)], 0)
    par = np.concatenate([res[c]["o_par"] for c in range(NC_)], 0)
    rel = np.concatenate([res[c]["o_rel"].T for c in range(NC_)], 0)
    jj = np.arange(L + 1)
    tt_ = np.arange(L)
    par = np.where(jj[None, :] <= tt_[:, None], par,
                   np.float32(NEG)).astype(np.float32)
    return cls_logits, par, rel


def time_device(inputs, iters=6):
    """Best-effort device execution timing: inputs staged on device once,
    min wall time of repeated sharded executions (includes PJRT dispatch)."""
    import time
    import jax
    import numpy as _np
    from jax.sharding import Mesh, PartitionSpec
    from jax.experimental.shard_map import shard_map
    from concourse import bass2jax as b2j

    nc = _get_prog()
    in_maps = _shard_inputs(inputs)
    b2j.install_neuronx_cc_hook()
    partition_name = nc.partition_id_tensor.name if nc.partition_id_tensor else None
    import concourse.mybir as _mb
    in_names, out_names, out_avals, zero_outs = [], [], [], []
    for alloc in nc.m.functions[0].allocations:
        if not isinstance(alloc, _mb.MemoryLocationSet):
            continue
        name = alloc.memorylocations[0].name
        if alloc.kind == "ExternalInput":
            if name != partition_name:
                in_names.append(name)
        elif alloc.kind == "ExternalOutput":
            shape = tuple(alloc.tensor_shape)
            dtype = _mb.dt.np(alloc.dtype)
            out_names.append(name)
            out_avals.append(jax.core.ShapedArray(shape, dtype))
            zero_outs.append(_np.zeros(shape, dtype))
    n_params = len(in_names)
    n_outs = len(out_avals)
    in_names_all = in_names + out_names
    if partition_name is not None:
        in_names_all.append(partition_name)

    def _body(*args):
        operands = list(args)
        if partition_name is not None:
            operands.append(b2j.partition_id_tensor())
        outs = b2j._bass_exec_p.bind(
            *operands, out_avals=tuple(out_avals), in_names=tuple(in_names_all),
            out_names=tuple(out_names), lowering_input_output_aliases=(),
            sim_require_finite=True, sim_require_nnan=True, nc=nc)
        return tuple(outs)

    devices = jax.devices()[:NC_]
    mesh = Mesh(_np.asarray(devices), ("core",))
    in_specs = (PartitionSpec("core"),) * (n_params + n_outs)
    out_specs = (PartitionSpec("core"),) * n_outs
    sharded = jax.jit(shard_map(_body, mesh=mesh, in_specs=in_specs,
                                out_specs=out_specs, check_rep=False),
                      keep_unused=True)
    concat_in = [_np.concatenate([_np.asarray(in_maps[c][nm]) for c in range(NC_)], 0)
                 for nm in in_names]
    concat_zero = [_np.concatenate([z] * NC_, 0) for z in zero_outs]
    args = [jax.device_put(a) for a in concat_in + concat_zero]
    best = float("inf")
    for _ in range(iters):
        t0 = time.perf_counter()
        outs = sharded(*args)
        jax.block_until_ready(outs)
        best = min(best, time.perf_counter() - t0)
    return best


# revision 26
# speedup vs baseline: 1.1716x; 1.0012x over previous
"""Trainium2 Bass kernel for nn_DSPSModel (8-core SPMD).

Sharding:
- Encoder: tokens sharded 256/core for LN/residual (sequence parallel);
  attention heads (2/core) and FFN columns (512/core) tensor parallel;
  AllGather h.T before QKV/FFN, ReduceScatter after Wo/W2.
- GRU scan -> halo'd Jacobi fixed-point iteration: each core iterates a
  384-token block (128-token halo) with batched matmul sweeps in
  transposed layout; error contracts ~0.72x per sweep; no cross-core
  traffic during sweeps.  h_{-1}=0 boundary enforced via cmask0.
- Downstream (cls / parent scores / relation) sharded over query rows;
  one fused AllGather shares h_seq rows + root + keys.T + PM.T blocks.
- Big matmuls in float32r (tf32-like precision, full PE rate at N>=256).
"""
import sys
sys.path.insert(0, "/opt/trn_rl_repo")
import numpy as np
import concourse.bass as bass
import concourse.mybir as mybir
import concourse.tile as tile
from contextlib import ExitStack
from concourse import bacc
from concourse.bass_utils import run_bass_kernel_spmd
from concourse.masks import make_identity

F32 = mybir.dt.float32
F32R = mybir.dt.float32r
I32 = mybir.dt.int32
AX = mybir.AxisListType
OP = mybir.AluOpType
AF = mybir.ActivationFunctionType

L, D, H, NL, C, R = 2048, 1024, 16, 4, 64, 32
DH = D // H
NC_ = 8
TOK = L // NC_            # 256 tokens per core
BLK = TOK + 128           # 384-token GRU jacobi block (128-token halo)
import os as _os
KSWEEPS = int(_os.environ.get("GRU_SWEEPS", "32"))
NL_BUILD = int(_os.environ.get("NL_BUILD", "4"))
EPS_PAR = 1e-8
LN_EPS = 1e-5
SCALE = 1.0 / float(np.sqrt(DH))
NEG = -1e9

# AG4 per-rank block layout (flat fp32 elements, viewed as rows of 1024):
HSEQ_E = 0                       # h_seq own rows   [256, 1024]
ROOT_E = TOK * D                 # root row         [1, 1024]
KEYS_E = (TOK + 1) * D           # keys.T ext       8 blocks of [128, 257]
PM_E = KEYS_E + 8 * 128 * 257    # PM.T own         [64, 256]
AG4_ROWS = (PM_E + C * TOK) // D  # = 530

ts = bass.ts


def _ln_tile(nc, sb, x_ap, tag, eps_ap=None):
    """LayerNorm along the free dim of a [128, D] fp32 tile, in place."""
    s = sb.tile([128, 1], F32, tag=f"{tag}_s")
    nc.vector.tensor_reduce(s[:], x_ap, AX.X, OP.add)
    m = sb.tile([128, 1], F32, tag=f"{tag}_m")
    nc.scalar.mul(m[:], s[:], 1.0 / D)
    sq = sb.tile([128, D], F32, tag=f"{tag}_sq")
    ss = sb.tile([128, 1], F32, tag=f"{tag}_ss")
    nc.scalar.activation(sq[:], x_ap, AF.Square, accum_out=ss[:])
    m2 = sb.tile([128, 1], F32, tag=f"{tag}_m2")
    nc.scalar.square(m2[:], m[:])
    v = sb.tile([128, 1], F32, tag=f"{tag}_v")
    nc.vector.tensor_scalar(v[:], ss[:], 1.0 / D, m2[:], OP.mult, OP.subtract)
    sd = sb.tile([128, 1], F32, tag=f"{tag}_sd")
    nc.scalar.activation(sd[:], v[:], AF.Sqrt, bias=eps_ap)
    rstd = sb.tile([128, 1], F32, tag=f"{tag}_r")
    nc.vector.reciprocal(rstd[:], sd[:])
    nc.vector.tensor_scalar(x_ap, x_ap, m[:], rstd[:], OP.subtract, OP.mult)


def build_program():
    nc = bacc.Bacc("TRN2", target_bir_lowering=False, debug=False,
                   num_devices=NC_, enable_asserts=False)

    # ---------------- DRAM I/O ----------------
    x_in = nc.dram_tensor("x_c", [TOK, D], F32, kind="ExternalInput")
    wq, wk, wv, wo, w1, w2 = [], [], [], [], [], []
    for l in range(NL):
        wq.append(nc.dram_tensor(f"wqT_{l}", [D, 128], F32R, kind="ExternalInput"))
        wk.append(nc.dram_tensor(f"wkT_{l}", [D, 128], F32R, kind="ExternalInput"))
        wv.append(nc.dram_tensor(f"wvT_{l}", [D, 128], F32R, kind="ExternalInput"))
        wo.append(nc.dram_tensor(f"woT_{l}", [128, D], F32R, kind="ExternalInput"))
        w1.append(nc.dram_tensor(f"w1T_{l}", [D, 512], F32R, kind="ExternalInput"))
        w2.append(nc.dram_tensor(f"w2T_{l}", [512, D], F32R, kind="ExternalInput"))
    clswT = nc.dram_tensor("clswT", [D, C], F32R, kind="ExternalInput")
    mcp_in = nc.dram_tensor("mcp", [C + 2, C], F32R, kind="ExternalInput")
    mcp_last = nc.dram_tensor("mcp_last", [C, 1], F32R, kind="ExternalInput")
    wihT = nc.dram_tensor("wihT", [D, 3 * D], F32R, kind="ExternalInput")
    whhT_in = nc.dram_tensor("whhT", [D, 3 * D], F32R, kind="ExternalInput")
    wqfT = nc.dram_tensor("wqfT", [D, D], F32R, kind="ExternalInput")
    wkfT = nc.dram_tensor("wkfT", [D, D], F32R, kind="ExternalInput")
    rw1T = nc.dram_tensor("rw1T", [2 * D, D], F32R, kind="ExternalInput")
    rw2T = nc.dram_tensor("rw2T", [D, R], F32R, kind="ExternalInput")
    maskb = nc.dram_tensor("maskb", [TOK, L + 1], F32, kind="ExternalInput")
    haloofs = nc.dram_tensor("haloofs", [128, 8], I32, kind="ExternalInput")
    pvofs = nc.dram_tensor("pvofs", [128, 2], I32, kind="ExternalInput")
    cmask0 = nc.dram_tensor("cmask0", [128, 1], F32, kind="ExternalInput")

    o_cls = nc.dram_tensor("o_cls", [C, TOK], F32, kind="ExternalOutput")
    o_par = nc.dram_tensor("o_par", [TOK, L + 1], F32, kind="ExternalOutput")
    o_rel = nc.dram_tensor("o_rel", [R, TOK], F32, kind="ExternalOutput")

    # internal DRAM bounces
    agh_in = [nc.dram_tensor(f"agh_in{i}", [D, TOK], F32R) for i in range(2 * NL)]
    agh_out = [nc.dram_tensor(f"agh_out{i}", [NC_, D, TOK], F32R, addr_space="Shared")
               for i in range(2 * NL)]
    rs_in = [nc.dram_tensor(f"rs_in{i}", [L, D], F32) for i in range(2 * NL)]
    rs_out = [nc.dram_tensor(f"rs_out{i}", [TOK, D], F32) for i in range(2 * NL)]
    root_b = nc.dram_tensor("root_b", [D], F32)
    root_o = nc.dram_tensor("root_o", [D], F32, addr_space="Shared")
    agx_in = nc.dram_tensor("agx_in", [D, 129], F32R)
    agx_out = nc.dram_tensor("agx_out", [NC_, D, 129], F32R, addr_space="Shared")
    xstar_d = nc.dram_tensor("xstar_d", [D, TOK], F32R)
    ag4_in = nc.dram_tensor("ag4_in", [AG4_ROWS, D], F32)
    ag4_out = nc.dram_tensor("ag4_out", [NC_, AG4_ROWS, D], F32, addr_space="Shared")

    rg = [list(range(NC_))]
    ag4i_flat = ag4_in[:].rearrange("a b -> (a b)")

    with tile.TileContext(nc) as tc:
        with (
            tc.tile_pool(name="const", bufs=1) as cpool,
            tc.tile_pool(name="psumA", bufs=3, space="PSUM") as ps,
            tc.tile_pool(name="psumB", bufs=1, space="PSUM") as psb,
        ):
            ident = cpool.tile([128, 128], F32)
            make_identity(nc, ident[:])
            eps_ln = cpool.tile([128, 1], F32)
            nc.vector.memset(eps_ln[:], LN_EPS)
            eps_par = cpool.tile([128, 1], F32)
            nc.vector.memset(eps_par[:], EPS_PAR)

            _enc_stack = ExitStack()
            ep = _enc_stack.enter_context(tc.tile_pool(name="enc", bufs=1))
            ew = _enc_stack.enter_context(tc.tile_pool(name="encw", bufs=2))
            h_sh = ep.tile([128, 2, D], F32, tag="h_sh")
            hTfull = ep.tile([128, NC_, L], F32R, tag="hTfull")
            hT_sh = ep.tile([128, NC_, TOK], F32R, tag="hT_sh")

            # ---------- fuse LN ----------
            for m in range(2):
                nc.sync.dma_start(h_sh[:, m, :], x_in[ts(m, 128), :])
                _ln_tile(nc, ew, h_sh[:, m, :], "ln", eps_ln[:])

            def transpose_shard(dst_sb):
                for m in range(2):
                    for k in range(8):
                        pt = ps.tile([128, 128], F32, tag="mm")
                        nc.tensor.transpose(pt[:], h_sh[:, m, ts(k, 128)], ident[:])
                        nc.scalar.copy(dst_sb[:, k, ts(m, 128)], pt[:])

            def ag_layer(i):
                nc.sync.dma_start(agh_in[i][:].rearrange("(k p) t -> p k t", p=128),
                                  hT_sh[:])
                nc.gpsimd.collective_compute(
                    "AllGather", OP.bypass, ins=[agh_in[i][:]], outs=[agh_out[i][:]],
                    replica_groups=rg)

            def load_hTfull(i):
                for k in range(8):
                    nc.sync.dma_start(
                        hTfull[:, k, :].rearrange("p (r t) -> p r t", r=NC_),
                        agh_out[i][:, ts(k, 128), :].rearrange("r p t -> p r t"))

            transpose_shard(hT_sh)
            ag_layer(0)

            # ---------------- encoder layers ----------------
            for l in range(NL_BUILD):
                with tc.tile_pool(name=f"lay{l}", bufs=1) as lp, \
                     tc.tile_pool(name=f"l2a_{l}", bufs=2) as lw2, \
                     tc.tile_pool(name=f"l2b_{l}", bufs=2) as lwb:
                    wq_s = lp.tile([128, 8, 128], F32R, tag="wq")
                    wk_s = lp.tile([128, 8, 128], F32R, tag="wk")
                    wv_s = lp.tile([128, 8, 128], F32R, tag="wv")
                    wo_h0 = lp.tile([64, D], F32R, tag="wo0")
                    wo_h1 = lp.tile([64, D], F32R, tag="wo1")
                    wo_h = [wo_h0, wo_h1]
                    nc.sync.dma_start(wq_s[:], wq[l][:].rearrange("(k p) m -> p k m", p=128))
                    nc.sync.dma_start(wk_s[:], wk[l][:].rearrange("(k p) m -> p k m", p=128))
                    nc.sync.dma_start(wv_s[:], wv[l][:].rearrange("(k p) m -> p k m", p=128))
                    nc.sync.dma_start(wo_h0[:], wo[l][0:64, :])
                    nc.sync.dma_start(wo_h1[:], wo[l][64:128, :])
                    load_hTfull(2 * l)

                    # k.T (both heads, all tokens) + v_ext
                    kT = lp.tile([128, L], F32R, tag="kT")
                    v_ext = lp.tile([128, 16, 132], F32R, tag="v_ext")
                    nc.vector.memset(v_ext[:].bitcast(F32), 1.0)
                    for nchk in range(4):
                        pq = ps.tile([128, 512], F32, tag="mm")
                        for k in range(8):
                            nc.tensor.matmul(pq[:], wk_s[:, k, :],
                                             hTfull[:, k, ts(nchk, 512)],
                                             start=(k == 0), stop=(k == 7))
                        nc.scalar.copy(kT[:, ts(nchk, 512)], pq[:])
                        pq2 = ps.tile([128, 512], F32, tag="mm")
                        for k in range(8):
                            nc.tensor.matmul(pq2[:], wv_s[:, k, :],
                                             hTfull[:, k, ts(nchk, 512)],
                                             start=(k == 0), stop=(k == 7))
                        vev = lw2.tile([128, 512], F32, tag="ev")
                        nc.scalar.copy(vev[:], pq2[:])
                        for i4 in range(4):
                            kt = 4 * nchk + i4
                            pv = ps.tile([128, 128], F32, tag="mm")
                            nc.tensor.transpose(pv[:], vev[:, ts(i4, 128)], ident[:])
                            nc.scalar.copy(v_ext[:, kt, 0:64], pv[:, 0:64])
                            nc.scalar.copy(v_ext[:, kt, 66:130], pv[:, 64:128])

                    # attention + fused Wo, per 512-token query chunk
                    for qc in range(4):
                        qTc = lw2.tile([128, 512], F32R, tag="qTc")
                        pq = ps.tile([128, 512], F32, tag="mm")
                        for k in range(8):
                            nc.tensor.matmul(pq[:], wq_s[:, k, :],
                                             hTfull[:, k, ts(qc, 512)],
                                             start=(k == 0), stop=(k == 7))
                        nc.scalar.copy(qTc[:], pq[:])
                        oTc0 = lw2.tile([64, 512], F32R, tag="oTc0")
                        oTc1 = lw2.tile([64, 512], F32R, tag="oTc1")
                        oTc = [oTc0, oTc1]
                        for h in range(2):
                            po = psb.tile([66, 512], F32, tag="pv")
                            for half in range(4):
                                es = [lp.tile([128, 512], F32R, tag=f"es{i}",
                                              name=f"es{i}") for i in range(4)]
                                for i in range(4):
                                    kt = 4 * half + i
                                    pst = ps.tile([128, 512], F32, tag="mm")
                                    nc.tensor.matmul(
                                        pst[:], kT[ts(h, 64), ts(kt, 128)],
                                        qTc[ts(h, 64), :], start=True, stop=True)
                                    nc.scalar.activation(es[i][:], pst[:], AF.Exp,
                                                         scale=SCALE)
                                for i in range(4):
                                    kt = 4 * half + i
                                    nc.tensor.matmul(po[:], v_ext[:, kt, ts(h, 66)],
                                                     es[i][:], start=(kt == 0),
                                                     stop=(kt == 15))
                            zhi = lw2.tile([66, 512], F32, tag="zhi")
                            nc.scalar.copy(zhi[64:65, :], po[64:65, :])
                            zr = lw2.tile([1, 512], F32, tag="zr")
                            nc.sync.dma_start(zr[:], zhi[64:65, :])
                            nc.vector.reciprocal(zr[:], zr[:])
                            zrb = lw2.tile([64, 512], F32, tag="zrb")
                            nc.gpsimd.partition_broadcast(zrb[:], zr[:])
                            nc.vector.tensor_tensor(oTc[h][:], po[0:64, :], zrb[:],
                                                    OP.mult)
                        for tt4 in range(4):
                            tt = 4 * qc + tt4
                            for nck in range(2):
                                pa = ps.tile([128, 512], F32, tag="mm")
                                for h in range(2):
                                    nc.tensor.matmul(pa[:], oTc[h][:, ts(tt4, 128)],
                                                     wo_h[h][:, ts(nck, 512)],
                                                     start=(h == 0), stop=(h == 1))
                                sa = lw2.tile([128, 512], F32, tag="ev")
                                nc.vector.tensor_copy(sa[:], pa[:])
                                nc.sync.dma_start(
                                    rs_in[2 * l][ts(tt, 128), ts(nck, 512)], sa[:])
                    nc.gpsimd.collective_compute(
                        "ReduceScatter", OP.add, ins=[rs_in[2 * l][:]],
                        outs=[rs_out[2 * l][:]], replica_groups=rg)
                    for m in range(2):
                        radd = ew.tile([128, D], F32, tag="radd")
                        nc.sync.dma_start(radd[:], rs_out[2 * l][ts(m, 128), :])
                        nc.vector.tensor_tensor(h_sh[:, m, :], h_sh[:, m, :],
                                                radd[:], OP.add)
                        _ln_tile(nc, ew, h_sh[:, m, :], "ln", eps_ln[:])
                    transpose_shard(hT_sh)
                    ag_layer(2 * l + 1)
                    load_hTfull(2 * l + 1)

                    # FFN (W1/W2 streamed; f1T holds a quarter of tokens)
                    f1T = lp.tile([128, 4, L // 4], F32R, tag="f1T")
                    for tc2 in range(4):
                        for m in range(4):
                            w1c = lwb.tile([128, 8, 128], F32R, tag="w1c")
                            nc.sync.dma_start(
                                w1c[:],
                                w1[l][:, ts(m, 128)].rearrange("(k p) f -> p k f", p=128))
                            pf = ps.tile([128, 512], F32, tag="mm")
                            for k in range(8):
                                nc.tensor.matmul(pf[:], w1c[:, k, :],
                                                 hTfull[:, k, ts(tc2, 512)],
                                                 start=(k == 0), stop=(k == 7))
                            nc.scalar.activation(f1T[:, m, :], pf[:], AF.Gelu)
                        for nck in range(2):
                            w2c = lwb.tile([128, 4, 512], F32R, tag="w2c")
                            nc.sync.dma_start(
                                w2c[:],
                                w2[l][:, ts(nck, 512)].rearrange("(k p) f -> p k f", p=128))
                            for tt4 in range(4):
                                tt = 4 * tc2 + tt4
                                pf2 = ps.tile([128, 512], F32, tag="mm")
                                for k4 in range(4):
                                    nc.tensor.matmul(pf2[:], f1T[:, k4, ts(tt4, 128)],
                                                     w2c[:, k4, :],
                                                     start=(k4 == 0), stop=(k4 == 3))
                                sf = lw2.tile([128, 512], F32, tag="ev")
                                nc.vector.tensor_copy(sf[:], pf2[:])
                                nc.sync.dma_start(
                                    rs_in[2 * l + 1][ts(tt, 128), ts(nck, 512)], sf[:])
                    nc.gpsimd.collective_compute(
                        "ReduceScatter", OP.add, ins=[rs_in[2 * l + 1][:]],
                        outs=[rs_out[2 * l + 1][:]], replica_groups=rg)
                    for m in range(2):
                        radd = ew.tile([128, D], F32, tag="radd")
                        nc.sync.dma_start(radd[:], rs_out[2 * l + 1][ts(m, 128), :])
                        nc.vector.tensor_tensor(h_sh[:, m, :], h_sh[:, m, :],
                                                radd[:], OP.add)
                        _ln_tile(nc, ew, h_sh[:, m, :], "ln", eps_ln[:])
                    if l < NL_BUILD - 1:
                        transpose_shard(hT_sh)
                        ag_layer(2 * l + 2)

            # ============ x_star: h_sh (normal), xstarT (transposed) ============
            transpose_shard(hT_sh)
            nc.sync.dma_start(xstar_d[:].rearrange("(k p) t -> p k t", p=128),
                              hT_sh[:])
            nc.sync.dma_start(agx_in[:, 0:128].rearrange("(k p) t -> p k t", p=128),
                              hT_sh[:, :, 128:256])
            _enc_stack.close()
            _mid_stack = ExitStack()
            pp = _mid_stack.enter_context(tc.tile_pool(name="mid", bufs=1))
            mw = _mid_stack.enter_context(tc.tile_pool(name="midw", bufs=2))
            xstarT = pp.tile([128, NC_, TOK], F32R, tag="xstarT")
            nc.sync.dma_start(xstarT[:],
                              xstar_d[:].rearrange("(k p) t -> p k t", p=128))

            # ---- cls logits + softmax ----
            clsw_s = pp.tile([128, 8, C], F32R, tag="clsw")
            nc.sync.dma_start(clsw_s[:], clswT[:].rearrange("(k p) m -> p k m", p=128))
            pc = ps.tile([C, TOK], F32, tag="mm")
            for k in range(8):
                nc.tensor.matmul(pc[:], clsw_s[:, k, :], xstarT[:, k, :],
                                 start=(k == 0), stop=(k == 7))
            cls_sb = mw.tile([C, TOK], F32, tag="cls_sb")
            nc.vector.tensor_copy(cls_sb[:], pc[:])
            nc.sync.dma_start(o_cls[:], cls_sb[:])
            e_cls = pp.tile([C, TOK], F32R, tag="e_cls")
            nc.scalar.activation(e_cls[:], pc[:], AF.Exp)
            ones64 = cpool.tile([C, 1], F32R)
            nc.vector.memset(ones64[:].bitcast(F32), 1.0)
            pz = ps.tile([1, TOK], F32, tag="mm")
            nc.tensor.matmul(pz[:], ones64[:], e_cls[:], start=True, stop=True)
            zrc = mw.tile([1, TOK], F32, tag="zrc")
            nc.vector.reciprocal(zrc[:], pz[:])
            zrcb = mw.tile([C, TOK], F32, tag="zrcb")
            nc.gpsimd.partition_broadcast(zrcb[:], zrc[:])
            cprobT = pp.tile([C, TOK], F32R, tag="cprobT")
            nc.vector.tensor_tensor(cprobT[:], e_cls[:], zrcb[:], OP.mult)

            # ---- root mean + halo via one AllGather (agx col 128 = rowsum) ----
            rsum = mw.tile([128, 8, 1], F32, tag="rsum")
            nc.vector.tensor_reduce(rsum[:], xstarT[:].bitcast(F32), AX.X, OP.add)
            nc.sync.dma_start(
                agx_in[:, 128:129].bitcast(F32).rearrange("(k p) o -> p k o", p=128),
                rsum[:])
            nc.gpsimd.collective_compute("AllGather", OP.bypass, ins=[agx_in[:]],
                                         outs=[agx_out[:]], replica_groups=rg)
            rparts = mw.tile([128, 8, 8], F32, tag="rparts")
            for r_ in range(NC_):
                nc.sync.dma_start(
                    rparts[:, :, r_:r_ + 1],
                    agx_out[r_, :, 128:129].bitcast(F32)
                    .rearrange("(k p) o -> p k o", p=128))
            rsum8 = mw.tile([128, 8, 1], F32, tag="rsum8")
            nc.vector.tensor_reduce(rsum8[:], rparts[:], AX.X, OP.add)
            root_s = pp.tile([128, 8, 1], F32R, tag="root_s")
            nc.scalar.activation(root_s[:, :, 0], rsum8[:, :, 0], AF.Copy, scale=1.0 / L)
            rtmp = mw.tile([128, 8], F32, tag="rtmp")
            nc.scalar.activation(rtmp[:], rsum8[:, :, 0], AF.Copy, scale=1.0 / L)
            hofs_s = pp.tile([128, 8], I32, tag="hofs")
            nc.sync.dma_start(hofs_s[:], haloofs[:])
            xhalo = pp.tile([128, 8, 128], F32R, tag="xhalo")
            for k in range(8):
                nc.gpsimd.indirect_dma_start(
                    out=xhalo[:, k, :], out_offset=None,
                    in_=agx_out[:].rearrange("r d t -> (r d) t"),
                    in_offset=bass.IndirectOffsetOnAxis(ap=hofs_s[:, k:k + 1], axis=0))

            # ============ GRU ============
            with tc.tile_pool(name="gruo", bufs=1) as go:
                hts = [go.tile([128, 8, BLK + 1], F32R, tag=f"ht{i}", name=f"ht{i}")
                       for i in range(2)]
                nc.vector.memset(hts[0][:].bitcast(F32), 0.0)
                nc.vector.memset(hts[1][:].bitcast(F32), 0.0)
                htf = hts[KSWEEPS % 2]

                with tc.tile_pool(name="gru", bufs=1) as gp, \
                     tc.tile_pool(name="gsc", bufs=2) as gsc, \
                     tc.tile_pool(name="gw", bufs=1) as gw:
                    giT = gp.tile([128, 24, BLK], F32, tag="giT")
                    for jb in range(8):
                        wcol = gw.tile([128, 8, 384], F32R, tag="wcol")
                        nc.sync.dma_start(
                            wcol[:],
                            wihT[:, ts(jb, 384)].rearrange("(k p) m -> p k m", p=128))
                        for mj in range(3):
                            pg = ps.tile([128, BLK], F32, tag="mm")
                            for k in range(8):
                                nc.tensor.matmul(pg[:, 0:128], wcol[:, k, ts(mj, 128)],
                                                 xhalo[:, k, :],
                                                 start=(k == 0), stop=(k == 7))
                            for k in range(8):
                                nc.tensor.matmul(pg[:, 128:BLK], wcol[:, k, ts(mj, 128)],
                                                 xstarT[:, k, :],
                                                 start=(k == 0), stop=(k == 7))
                            nc.scalar.copy(giT[:, 3 * jb + mj, :], pg[:])

                    whh_s = gp.tile([128, 8, 3 * D], F32R, tag="whh_s")
                    nc.sync.dma_start(whh_s[:],
                                      whhT_in[:].rearrange("(k p) m -> p k m", p=128))
                    cm0 = cpool.tile([128, 1], F32)
                    nc.sync.dma_start(cm0[:], cmask0[:])

                    for s in range(KSWEEPS):
                        ha, hb = hts[s % 2], hts[(s + 1) % 2]
                        for kk in range(8):
                            pgs = []
                            for g in range(3):
                                mj = 8 * g + kk
                                pgg = psb.tile([128, BLK], F32, tag=f"gh{g}", name=f"gh{g}")
                                for k in range(8):
                                    nc.tensor.matmul(pgg[:], whh_s[:, k, ts(mj, 128)],
                                                     ha[:, k, 0:BLK],
                                                     start=(k == 0), stop=(k == 7))
                                pgs.append(pgg)
                            r_t = gsc.tile([128, BLK], F32, tag="r_t")
                            nc.vector.tensor_tensor(r_t[:], pgs[0][:], giT[:, kk, :],
                                                    OP.add)
                            nc.scalar.activation(r_t[:], r_t[:], AF.Sigmoid)
                            z_t = gsc.tile([128, BLK], F32, tag="z_t")
                            nc.vector.tensor_tensor(z_t[:], pgs[1][:],
                                                    giT[:, 8 + kk, :], OP.add)
                            nc.scalar.activation(z_t[:], z_t[:], AF.Sigmoid)
                            n_t = gsc.tile([128, BLK], F32, tag="n_t")
                            nc.vector.tensor_tensor(n_t[:], r_t[:], pgs[2][:], OP.mult)
                            nc.vector.tensor_tensor(n_t[:], n_t[:],
                                                    giT[:, 16 + kk, :], OP.add)
                            nc.scalar.activation(n_t[:], n_t[:], AF.Tanh)
                            d_t = gsc.tile([128, BLK], F32, tag="d_t")
                            nc.vector.tensor_tensor(d_t[:], ha[:, kk, 0:BLK].bitcast(F32),
                                                    n_t[:], OP.subtract)
                            nc.vector.tensor_tensor(d_t[:], z_t[:], d_t[:], OP.mult)
                            nc.vector.tensor_tensor(hb[:, kk, 1:BLK + 1], n_t[:],
                                                    d_t[:], OP.add)
                        nc.vector.tensor_scalar_mul(hb[:, :, 128:129],
                                                    hb[:, :, 128:129], cm0[:])

                # ---------- build AG4 block (gru pools closed) ----------
                with tc.tile_pool(name="post", bufs=1) as gp, \
                     tc.tile_pool(name="postw", bufs=2) as gw2:
                    hnorm = gp.tile([128, 2, D], F32, tag="hnorm")
                    for m in range(2):
                        for k in range(8):
                            pt = ps.tile([128, 128], F32, tag="mm")
                            nc.tensor.transpose(
                                pt[:],
                                htf[:, k, 129 + 128 * m:129 + 128 * (m + 1)].bitcast(F32),
                                ident[:])
                            nc.scalar.copy(hnorm[:, m, ts(k, 128)], pt[:])
                        nc.sync.dma_start(ag4_in[ts(m, 128), :], hnorm[:, m, :])
                    nc.sync.dma_start(
                        ag4_in[TOK:TOK + 1, :].rearrange("o (k p) -> p (k o)", p=128),
                        root_s[:, :, 0].bitcast(F32))
                    # keys.T ext: [root | h own] through Wk
                    krhs = gp.tile([128, 8, 260], F32R, tag="krhs")
                    nc.vector.memset(krhs[:].bitcast(F32), 0.0)
                    nc.scalar.copy(krhs[:, :, 0:1], root_s[:])
                    nc.scalar.copy(krhs[:, :, 1:257], htf[:, :, 129:BLK + 1])
                    wkf_s = gp.tile([128, 8, D], F32R, tag="wkf_s")
                    nc.sync.dma_start(wkf_s[:],
                                      wkfT[:].rearrange("(k p) m -> p k m", p=128))
                    for m in range(8):
                        pk = ps.tile([128, 260], F32, tag="mm")
                        for k in range(8):
                            nc.tensor.matmul(pk[:], wkf_s[:, k, ts(m, 128)], krhs[:, k, :],
                                             start=(k == 0), stop=(k == 7))
                        sk = gw2.tile([128, 257], F32, tag="sk")
                        nc.vector.tensor_copy(sk[:], pk[:, 0:257])
                        nc.sync.dma_start(
                            ag4i_flat[KEYS_E + m * 128 * 257:
                                      KEYS_E + (m + 1) * 128 * 257]
                            .rearrange("(p j) -> p j", p=128), sk[:])
                    # PM.T own
                    mcp_s = cpool.tile([C + 2, C], F32R)
                    nc.sync.dma_start(mcp_s[:], mcp_in[:])
                    ppT = gp.tile([C + 2, TOK], F32R, tag="ppT")
                    nc.vector.memset(ppT[C:C + 2, :].bitcast(F32), 0.0)
                    nc.scalar.copy(ppT[0:C, :], cprobT[:])
                    pm = ps.tile([C, TOK], F32, tag="mm")
                    nc.tensor.matmul(pm[:], mcp_s[:], ppT[:], start=True, stop=True)
                    spm = gw2.tile([C, TOK], F32, tag="spm")
                    nc.vector.tensor_copy(spm[:], pm[:])
                    nc.sync.dma_start(
                        ag4i_flat[PM_E:PM_E + C * TOK].rearrange("(p j) -> p j", p=C),
                        spm[:])
                    nc.gpsimd.collective_compute(
                        "AllGather", OP.bypass, ins=[ag4_in[:]], outs=[ag4_out[:]],
                        replica_groups=rg)

                # ============ downstream ============
                with tc.tile_pool(name="down", bufs=1) as dp, \
                     tc.tile_pool(name="dw", bufs=2) as dw:
                    qT_o = dp.tile([128, 8, TOK], F32R, tag="qT_o")
                    for m in range(8):
                        wqf_s = dw.tile([128, 8, 128], F32R, tag="wqf_s")
                        nc.sync.dma_start(
                            wqf_s[:],
                            wqfT[:, ts(m, 128)].rearrange("(k p) mm -> p k mm", p=128))
                        pq = ps.tile([128, TOK], F32, tag="mm")
                        for k in range(8):
                            nc.tensor.matmul(pq[:], wqf_s[:, k, :],
                                             htf[:, k, 129:BLK + 1],
                                             start=(k == 0), stop=(k == 7))
                        nc.scalar.copy(qT_o[:, m, :], pq[:])

                    def keys_sec(r):
                        return (ag4_out[r][:].rearrange("a b -> (a b)")
                                [KEYS_E:KEYS_E + 8 * 128 * 257]
                                .rearrange("(m p j) -> p m j", p=128, j=257))

                    keysT = dp.tile([128, 8, L + 1], F32R, tag="keysT")
                    nc.sync.dma_start(keysT[:, :, 0:1].bitcast(F32),
                                      keys_sec(0)[:, :, 0:1])
                    for r_ in range(NC_):
                        nc.sync.dma_start(
                            keysT[:, :, 1 + TOK * r_:1 + TOK * (r_ + 1)].bitcast(F32),
                            keys_sec(r_)[:, :, 1:257])
                    pmT = dp.tile([C, L + 1], F32R, tag="pmT")
                    nc.sync.dma_start(pmT[:, 0:1], mcp_last[:])
                    for r_ in range(NC_):
                        nc.sync.dma_start(
                            pmT[:, 1 + TOK * r_:1 + TOK * (r_ + 1)].bitcast(F32),
                            ag4_out[r_][:].rearrange("a b -> (a b)")
                            [PM_E:PM_E + C * TOK].rearrange("(p j) -> p j", p=C))

                    mb_s = dp.tile([128, 2, L + 1], F32, tag="mb_s")
                    nc.sync.dma_start(mb_s[:],
                                      maskb[:].rearrange("(m p) j -> p m j", p=128))
                    nchunks = [(0, 512), (512, 512), (1024, 512), (1536, 512), (2047, 2)]
                    for m in range(2):
                        for (j0, jw) in nchunks:
                            pss = ps.tile([128, 512], F32, tag="mm")
                            for k in range(8):
                                nc.tensor.matmul(pss[:, 0:jw], qT_o[:, k, ts(m, 128)],
                                                 keysT[:, k, j0:j0 + jw],
                                                 start=(k == 0), stop=(k == 7))
                            psp = psb.tile([128, 512], F32, tag="pr")
                            nc.tensor.matmul(psp[:, 0:jw], cprobT[:, ts(m, 128)],
                                             pmT[:, j0:j0 + jw], start=True, stop=True)
                            lp_t = dw.tile([128, 512], F32, tag="lp_t")
                            nc.scalar.activation(lp_t[:, 0:jw], psp[:, 0:jw], AF.Ln,
                                                 bias=eps_par[:])
                            nc.vector.tensor_tensor(lp_t[:, 0:jw], lp_t[:, 0:jw],
                                                    pss[:, 0:jw], OP.add)
                            nc.vector.tensor_tensor(lp_t[:, 0:jw], lp_t[:, 0:jw],
                                                    mb_s[:, m, j0:j0 + jw], OP.add)
                            nc.sync.dma_start(o_par[ts(m, 128), j0:j0 + jw],
                                              lp_t[:, 0:jw])

                    # relation head
                    pvo_s = dp.tile([128, 2], I32, tag="pvo_s")
                    nc.sync.dma_start(pvo_s[:], pvofs[:])
                    pvn = dp.tile([128, 2, D], F32, tag="pvn")
                    for m in range(2):
                        nc.gpsimd.indirect_dma_start(
                            out=pvn[:, m, :], out_offset=None,
                            in_=ag4_out[:].rearrange("r b d -> (r b) d"),
                            in_offset=bass.IndirectOffsetOnAxis(
                                ap=pvo_s[:, m:m + 1], axis=0))
                    pvT = dp.tile([128, 8, TOK], F32R, tag="pvT")
                    for m in range(2):
                        for k in range(8):
                            pt = ps.tile([128, 128], F32, tag="mm")
                            nc.tensor.transpose(pt[:], pvn[:, m, ts(k, 128)], ident[:])
                            nc.scalar.copy(pvT[:, k, ts(m, 128)], pt[:])
                    rel1 = dp.tile([128, 8, TOK], F32R, tag="rel1")
                    rw1cols = []
                    for m in range(8):
                        rw1_s = dw.tile([128, 16, 128], F32R, tag="rw1_s")
                        nc.sync.dma_start(
                            rw1_s[:],
                            rw1T[:, ts(m, 128)].rearrange("(k p) mm -> p k mm", p=128))
                        pr1 = ps.tile([128, TOK], F32, tag="mm")
                        for k in range(16):
                            rhs = htf[:, k, 129:BLK + 1] if k < 8 else pvT[:, k - 8, :]
                            nc.tensor.matmul(pr1[:], rw1_s[:, k, :], rhs,
                                             start=(k == 0), stop=(k == 15))
                        nc.scalar.activation(rel1[:, m, :], pr1[:], AF.Gelu)
                    rw2_s = dw.tile([128, 8, R], F32R, tag="rw2_s")
                    nc.sync.dma_start(rw2_s[:],
                                      rw2T[:].rearrange("(k p) m -> p k m", p=128))
                    pr2 = ps.tile([R, TOK], F32, tag="mm")
                    for k in range(8):
                        nc.tensor.matmul(pr2[:], rw2_s[:, k, :], rel1[:, k, :],
                                         start=(k == 0), stop=(k == 7))
                    sr2 = dw.tile([R, TOK], F32, tag="sr2")
                    nc.vector.tensor_copy(sr2[:], pr2[:])
                    nc.sync.dma_start(o_rel[:], sr2[:])

            _mid_stack.close()

    nc.finalize()
    return nc


_PROG = None


def _get_prog():
    global _PROG
    if _PROG is None:
        _PROG = build_program()
    return _PROG


def _shard_inputs(inputs):
    f = lambda a: np.ascontiguousarray(np.asarray(a, dtype=np.float32))
    x = f(inputs["x"])
    Wqkv = f(inputs["enc_Wqkv"]); Wo = f(inputs["enc_Wo"])
    W1 = f(inputs["enc_W1"]); W2 = f(inputs["enc_W2"])
    clsW = f(inputs["cls_W"]); Mcp = f(inputs["M_cp"])
    Wih = f(inputs["gru_Wih"]); Whh = f(inputs["gru_Whh"])
    Wq = f(inputs["Wq"]); Wk = f(inputs["Wk"])
    rW1 = f(inputs["rel_W1"]); rW2 = f(inputs["rel_W2"])
    y = np.asarray(inputs["y_parent"]).astype(np.int64)

    shared = {
        "clswT": np.ascontiguousarray(clsW.T),
        "mcp": np.pad(Mcp, ((0, 1), (0, 0))),
        "mcp_last": np.ascontiguousarray(Mcp[C][:, None]),
        "wihT": np.ascontiguousarray(Wih.T),
        "whhT": np.ascontiguousarray(Whh.T),
        "wqfT": np.ascontiguousarray(Wq.T),
        "wkfT": np.ascontiguousarray(Wk.T),
        "rw1T": np.ascontiguousarray(rW1.T),
        "rw2T": np.ascontiguousarray(rW2.T),
    }
    in_maps = []
    rows = np.arange(TOK)
    jj = np.arange(L + 1)
    k_ = np.arange(8)
    p_ = np.arange(128)
    for c in range(NC_):
        m = dict(shared)
        sl = slice(128 * c, 128 * (c + 1))
        fl = slice(512 * c, 512 * (c + 1))
        m["x_c"] = np.ascontiguousarray(x[TOK * c:TOK * (c + 1)])
        for l in range(NL):
            Wq_l = Wqkv[l][0:D]; Wk_l = Wqkv[l][D:2 * D]; Wv_l = Wqkv[l][2 * D:3 * D]
            m[f"wqT_{l}"] = np.ascontiguousarray(Wq_l[sl].T)
            m[f"wkT_{l}"] = np.ascontiguousarray(Wk_l[sl].T)
            m[f"wvT_{l}"] = np.ascontiguousarray(Wv_l[sl].T)
            m[f"woT_{l}"] = np.ascontiguousarray(Wo[l][:, sl].T)
            m[f"w1T_{l}"] = np.ascontiguousarray(W1[l][fl].T)
            m[f"w2T_{l}"] = np.ascontiguousarray(W2[l][:, fl].T)
        t_glob = TOK * c + rows
        m["maskb"] = np.ascontiguousarray(
            np.where(jj[None, :] <= t_glob[:, None], 0.0, NEG).astype(np.float32))
        prev = (c - 1) % NC_
        m["haloofs"] = np.ascontiguousarray(
            (prev * D + (k_[None, :] * 128 + p_[:, None])).astype(np.int32))
        yc = y[TOK * c:TOK * (c + 1)]
        ycl = np.clip(yc, 0, L - 1)
        pvrow = (ycl // TOK) * AG4_ROWS + (ycl % TOK)
        pvrow = np.where(yc < 0, TOK, pvrow)  # -> root row of rank 0
        m["pvofs"] = np.ascontiguousarray(pvrow.reshape(2, 128).T.astype(np.int32))
        m["cmask0"] = np.full((128, 1), 0.0 if c == 0 else 1.0, np.float32)
        in_maps.append(m)
    return in_maps


def kernel(**inputs):
    prog = _get_prog()
    in_maps = _shard_inputs(inputs)
    res = run_bass_kernel_spmd(prog, in_maps, list(range(NC_))).results
    cls_logits = np.concatenate([res[c]["o_cls"].T for c in range(NC_)], 0)
    par = np.concatenate([res[c]["o_par"] for c in range(NC_)], 0)
    rel = np.concatenate([res[c]["o_rel"].T for c in range(NC_)], 0)
    jj = np.arange(L + 1)
    tt_ = np.arange(L)
    par = np.where(jj[None, :] <= tt_[:, None], par,
                   np.float32(NEG)).astype(np.float32)
    return cls_logits, par, rel


def time_device(inputs, iters=6):
    """Best-effort device execution timing: inputs staged on device once,
    min wall time of repeated sharded executions (includes PJRT dispatch)."""
    import time
    import jax
    import numpy as _np
    from jax.sharding import Mesh, PartitionSpec
    from jax.experimental.shard_map import shard_map
    from concourse import bass2jax as b2j

    nc = _get_prog()
    in_maps = _shard_inputs(inputs)
    b2j.install_neuronx_cc_hook()
    partition_name = nc.partition_id_tensor.name if nc.partition_id_tensor else None
    import concourse.mybir as _mb
    in_names, out_names, out_avals, zero_outs = [], [], [], []
    for alloc in nc.m.functions[0].allocations:
        if not isinstance(alloc, _mb.MemoryLocationSet):
            continue
        name = alloc.memorylocations[0].name
        if alloc.kind == "ExternalInput":
            if name != partition_name:
                in_names.append(name)
        elif alloc.kind == "ExternalOutput":
            shape = tuple(alloc.tensor_shape)
            dtype = _mb.dt.np(alloc.dtype)
            out_names.append(name)
            out_avals.append(jax.core.ShapedArray(shape, dtype))
            zero_outs.append(_np.zeros(shape, dtype))
    n_params = len(in_names)
    n_outs = len(out_avals)
    in_names_all = in_names + out_names
    if partition_name is not None:
        in_names_all.append(partition_name)

    def _body(*args):
        operands = list(args)
        if partition_name is not None:
            operands.append(b2j.partition_id_tensor())
        outs = b2j._bass_exec_p.bind(
            *operands, out_avals=tuple(out_avals), in_names=tuple(in_names_all),
            out_names=tuple(out_names), lowering_input_output_aliases=(),
            sim_require_finite=True, sim_require_nnan=True, nc=nc)
        return tuple(outs)

    devices = jax.devices()[:NC_]
    mesh = Mesh(_np.asarray(devices), ("core",))
    in_specs = (PartitionSpec("core"),) * (n_params + n_outs)
    out_specs = (PartitionSpec("core"),) * n_outs
    sharded = jax.jit(shard_map(_body, mesh=mesh, in_specs=in_specs,
                                out_specs=out_specs, check_rep=False),
                      keep_unused=True)
    concat_in = [_np.concatenate([_np.asarray(in_maps[c][nm]) for c in range(NC_)], 0)
                 for nm in in_names]
    concat_zero = [_np.concatenate([z] * NC_, 0) for z in zero_outs]
    args = [jax.device_put(a) for a in concat_in + concat_zero]
    best = float("inf")
    for _ in range(iters):
        t0 = time.perf_counter()
        outs = sharded(*args)
        jax.block_until_ready(outs)
        best = min(best, time.perf_counter() - t0)
    return best
